# revision 1
# baseline (speedup 1.0000x reference)
"""AnchorToAnchor fused kernel for 8 TRN2 NeuronCores.

Shards data-parallel over the batch axis N=8 (one batch element per core).
Per core the device graph computes:
  1. block-strided conv (BoxRegress) as 129 accumulated TensorE matmuls
     (bias folded in as a rank-1 update)
  2. tanh-regressed sample centers + bilinear gather offsets/weights
  3. bilinear sampling via indirect DMA gathers from the (host-transposed)
     feature map, combined with per-partition-scalar DVE ops
  4. two anchor-to-anchor relation (softmax attention) passes with groups
     (anchor, channel) on partitions and the K x K score matrix in the free
     dimension. ScalarE expands b per-j into fp16 so the DVE outer-product
     TT runs at its 2x perf mode; exp on ScalarE (fp16 in -> bf16 out);
     e*a multiply + 3 bf16 tree-add halvings + a short tensor_reduce give
     den/num (tensor_reduce has no DVE fast mode, tree-adds do); final
     num/den combine in fp32 with a fast approximate reciprocal.

Engine notes baked into this design (measured on HW): DVE is the bottleneck
(~0.96 GHz, fp32 TT 1x, 16-bit TT 2x, single-src up to 4x; broadcast APs
with a step-0 innermost dim force 1x); GPSIMD shares SBUF ports with DVE so
offloading bulk elementwise work there is a wash; ScalarE runs ~1 elem/cyc
at 1.2 GHz for any dtype and has its own port budget, so it carries the
broadcast-expands, exps and psum copies. Compute instructions can embed only
one semaphore wait; building with bacc.Bacc legalizes multi-wait cases via
event-semaphore instructions.

The host wrapper only reshapes/transposes inputs into device-friendly
layouts (pure permutations), runs the SPMD NEFF on cores 0-7, and
re-assembles the full output.
"""

import sys

for _p in ("/opt/trn_rl_repo",):
    if _p not in sys.path:
        sys.path.insert(0, _p)

import numpy as np

# Problem constants (hardcoded per the task spec).
N, C, H, W = 8, 256, 64, 64
A, BS = 9, 8
F = H // BS          # 8
K = F * F            # 64
M = A * N * K        # 4608
ALPHA = 0.1
G = A * C            # 2304 groups per core
GT = G // 128        # 18 group tiles
ST = 5               # sample tiles of 128 (576 samples -> 4.5, padded)
NS = A * K           # 576 samples per core

# fbw16 blob: bf16 element offsets (stored as f32 words, bitcast on device)
W_OFF = 0            # conv weights [128, 128*9] bf16
B_OFF = 1152         # bias row (row 0 only) [9] bf16
ONE_OFF = 1161       # ones row (row 0 only) [64] bf16
FB_OFF = 1226        # conv feature [128, 8192] bf16 (even offset)
NFB16E = FB_OFF + 8192   # 9418 bf16 elements
NFBW = NFB16E // 2       # 4709 f32 words

# rb blob column offsets (f32 words)
CT_OFF = 0           # c-tensor [128, 18*64] f32
A16_OFF = 1152       # bf16 a-tensor packed [128, 576]
A16H_OFF = 1728      # fp16 a-tensor packed [128, 576]
XC_OFF = 2304        # x centers [128, 5]
YC_OFF = 2309        # y centers [128, 5]
ID_OFF = 2314        # identity [128, 128] f32
NRB = 2442

_CACHE = {}


def _build_nc():
    import concourse.bass as bass
    import concourse.bacc as bacc
    import concourse.tile as tile
    from concourse import mybir

    f32 = mybir.dt.float32
    bf16 = mybir.dt.bfloat16
    f16 = mybir.dt.float16
    i32 = mybir.dt.int32
    Alu = mybir.AluOpType
    Act = mybir.ActivationFunctionType

    nc = bacc.Bacc(None)

    fbw = nc.declare_dram_parameter("fbw", [128, NFBW], f32, isOutput=False)
    rb = nc.declare_dram_parameter("rb", [128, NRB], f32, isOutput=False)
    fbt = nc.declare_dram_parameter("fbt", [H * W, C], f32, isOutput=False)
    out_d = nc.declare_dram_parameter("out", [G, K], f32, isOutput=True)

    with tile.TileContext(nc) as tc:
        singles = tc.alloc_tile_pool(name="singles", bufs=1)
        gpool = tc.alloc_tile_pool(name="gpool", bufs=2)
        relpool = tc.alloc_tile_pool(name="relpool", bufs=3)
        ecpool = tc.alloc_tile_pool(name="ecpool", bufs=2)
        small = tc.alloc_tile_pool(name="small", bufs=4)
        ppool = tc.alloc_tile_pool(name="ppool", bufs=2, space="PSUM")
        cpsum = tc.alloc_tile_pool(name="cpsum", bufs=1, space="PSUM")

        # ---- resident loads (two blobs; fbw split over 4 queues) -----------
        fbw_sb = singles.tile([128, NFBW], f32)
        splits = [0, FB_OFF // 2, FB_OFF // 2 + 1024, FB_OFF // 2 + 2048,
                  FB_OFF // 2 + 3072, NFBW]
        for q in range(5):
            nc.sync.dma_start(out=fbw_sb[:, splits[q]:splits[q + 1]],
                              in_=fbw[:, splits[q]:splits[q + 1]])
        rb_sb = singles.tile([128, NRB], f32)
        nc.sync.dma_start(out=rb_sb[:, :NRB // 2], in_=rb[:, :NRB // 2])
        nc.sync.dma_start(out=rb_sb[:, NRB // 2:], in_=rb[:, NRB // 2:])

        # DVE pre-touch of the rb blob: its single DMA wait lands here so
        # later DVE consumers of rb carry no fresh semaphore.
        dve_touch = singles.tile([128, 1], f32)
        nc.vector.tensor_copy(out=dve_touch[:], in_=rb_sb[:, 0:1])

        fbw16 = fbw_sb[:].bitcast(bf16)                           # [128, 9418]
        at16_all = rb_sb[:, A16_OFF:A16_OFF + 576].bitcast(bf16)  # [128, 1152]
        a16h_all = rb_sb[:, A16H_OFF:A16H_OFF + 576].bitcast(f16)  # [128, 1152]
        ident = rb_sb[:, ID_OFF:ID_OFF + 128]
        xc_t = rb_sb[:, XC_OFF:XC_OFF + ST]
        yc_t = rb_sb[:, YC_OFF:YC_OFF + ST]

        # ---- conv (BoxRegress) in bf16 (4x PE rate), out [a, ij] -----------
        conv_ps = cpsum.tile([A, K], f32)
        for k in range(128):
            nc.tensor.matmul(
                out=conv_ps[:],
                lhsT=fbw16[:, W_OFF + 9 * k:W_OFF + 9 * k + 9],
                rhs=fbw16[:, FB_OFF + 64 * k:FB_OFF + 64 * k + 64],
                start=(k == 0),
                stop=False,
            )
        nc.tensor.matmul(
            out=conv_ps[:],
            lhsT=fbw16[0:1, B_OFF:B_OFF + A],
            rhs=fbw16[0:1, ONE_OFF:ONE_OFF + K],
            start=False,
            stop=True,
        )
        conv_s = singles.tile([A, K], f32)
        nc.scalar.copy(out=conv_s[:], in_=conv_ps[:])

        # reorg [a, ij] -> regs[(a ij) % 128, (a ij) // 128]
        regs = singles.tile([128, ST], f32)
        nc.scalar.memzero(regs[:])
        for t in range(ST):
            a0 = 2 * t
            nparts = 2 if t < 4 else 1
            nc.sync.dma_start(
                out=regs[0:64 * nparts, t:t + 1],
                in_=conv_s[a0:a0 + nparts, :],
            )

        # ---- centers, offsets, weights -------------------------------------
        th = small.tile([128, ST], f32)
        for t in range(ST):
            nc.scalar.activation(out=th[:, t:t + 1], in_=regs[:, t:t + 1],
                                 func=Act.Tanh)
        t8 = small.tile([128, ST], f32)
        nc.vector.tensor_scalar_mul(t8[:], th[:], ALPHA * BS)
        px = small.tile([128, ST], f32)
        py = small.tile([128, ST], f32)
        nc.vector.tensor_add(out=px[:], in0=t8[:], in1=xc_t)
        nc.vector.tensor_add(out=py[:], in0=t8[:], in1=yc_t)

        def floor_of(src, dst_f):
            ri = small.tile([128, ST], i32, tag="fl_i")
            nc.vector.tensor_copy(out=ri[:], in_=src[:])
            rf = small.tile([128, ST], f32, tag="fl_f")
            nc.vector.tensor_copy(out=rf[:], in_=ri[:])
            gt = small.tile([128, ST], f32, tag="fl_g")
            nc.vector.tensor_tensor(out=gt[:], in0=rf[:], in1=src[:],
                                    op=Alu.is_gt)
            nc.vector.tensor_sub(out=dst_f[:], in0=rf[:], in1=gt[:])

        x0f = small.tile([128, ST], f32)
        y0f = small.tile([128, ST], f32)
        floor_of(px, x0f)
        floor_of(py, y0f)
        wx = small.tile([128, ST], f32)
        wy = small.tile([128, ST], f32)
        nc.vector.tensor_sub(out=wx[:], in0=px[:], in1=x0f[:])
        nc.vector.tensor_sub(out=wy[:], in0=py[:], in1=y0f[:])
        ux = small.tile([128, ST], f32)
        uy = small.tile([128, ST], f32)
        nc.vector.tensor_scalar(out=ux[:], in0=wx[:], scalar1=-1.0, scalar2=1.0,
                                op0=Alu.mult, op1=Alu.add)
        nc.vector.tensor_scalar(out=uy[:], in0=wy[:], scalar1=-1.0, scalar2=1.0,
                                op0=Alu.mult, op1=Alu.add)

        o00f = small.tile([128, ST], f32)
        nc.vector.tensor_scalar(out=o00f[:], in0=y0f[:], scalar1=float(W),
                                scalar2=None, op0=Alu.mult)
        nc.vector.tensor_add(out=o00f[:], in0=o00f[:], in1=x0f[:])
        offs = []
        for d in (0.0, 1.0, 64.0, 65.0):
            of = small.tile([128, ST], f32, tag="of_f")
            if d == 0.0:
                nc.vector.tensor_copy(out=of[:], in_=o00f[:])
            else:
                nc.vector.tensor_scalar_add(of[:], o00f[:], d)
            oi = small.tile([128, ST], i32, tag=f"of_i{d}")
            nc.vector.tensor_copy(out=oi[:], in_=of[:])
            offs.append(oi)

        # ---- per sample-tile gather + bilinear; per anchor transpose + apps -
        wpairs = [(ux, uy), (wx, uy), (ux, wy), (wx, wy)]
        out1_sb = singles.tile([128, GT, K], f32)
        out116_sb = singles.tile([128, GT, K], bf16)
        out1h_sb = singles.tile([128, GT, K], f16)
        pend_g = None
        out_v = out_d.rearrange("(g p) k -> p g k", p=128)

        def app(a_h, a_b, b_f, o_f, o_b, o_h):
            # ScalarE materializes the per-j broadcast of b in fp16 so the
            # DVE outer-product TT has step-1 fp16 operands and runs at 2x.
            bexp = relpool.tile([128, K, K], f16, tag="bexp")
            nc.scalar.activation(out=bexp[:],
                                 in_=b_f.unsqueeze(2).to_broadcast([128, K, K]),
                                 func=Act.Copy)
            rel = relpool.tile([128, K, K], f16, tag="rel")
            nc.vector.tensor_tensor(
                out=rel[:],
                in0=a_h.unsqueeze(1).to_broadcast([128, K, K]),
                in1=bexp[:],
                op=Alu.mult,
            )
            ec = ecpool.tile([128, 2, K, K], bf16, tag="ec")
            nc.scalar.activation(out=ec[:, 0], in_=rel[:], func=Act.Exp)
            nc.vector.tensor_tensor(
                out=ec[:, 1],
                in0=ec[:, 0],
                in1=a_b.unsqueeze(1).to_broadcast([128, K, K]),
                op=Alu.mult,
            )
            # bf16 tree-adds run at 2x on DVE while tensor_reduce has no fast
            # mode; 3 halving levels then one small reduce is ~40% cheaper.
            t0 = ecpool.tile([128, 2, K, 32], bf16, tag="t0")
            nc.vector.tensor_tensor(out=t0[:], in0=ec[:, :, :, 0:32],
                                    in1=ec[:, :, :, 32:64], op=Alu.add)
            t1 = ecpool.tile([128, 2, K, 16], bf16, tag="t1")
            nc.vector.tensor_tensor(out=t1[:], in0=t0[:, :, :, 0:16],
                                    in1=t0[:, :, :, 16:32], op=Alu.add)
            t2 = ecpool.tile([128, 2, K, 8], bf16, tag="t2")
            nc.vector.tensor_tensor(out=t2[:], in0=t1[:, :, :, 0:8],
                                    in1=t1[:, :, :, 8:16], op=Alu.add)
            dn = small.tile([128, 2, K], f32, tag="dn")
            nc.vector.tensor_reduce(out=dn[:], in_=t2[:],
                                    axis=mybir.AxisListType.X, op=Alu.add)
            inv = small.tile([128, K], f32, tag="inv")
            nc.vector.reciprocal_approx_fast(out=inv[:], in_=dn[:, 0])
            r = small.tile([128, K], f32, tag="r")
            nc.vector.tensor_mul(out=r[:], in0=dn[:, 1], in1=inv[:])
            nc.vector.tensor_add(out=o_f[:], in0=r[:], in1=b_f[:])
            if o_b is not None:
                nc.scalar.copy(out=o_b[:], in_=o_f[:])
                # fp16 copy on DVE: app2's rel follows in-engine order, no
                # ScalarE round-trip on the serial app1->app2 chain
                nc.vector.tensor_copy(out=o_h[:], in_=o_f[:])

        for t in range(ST):
            vt = []
            for q in range(4):
                v = gpool.tile([128, C], f32, tag=f"v{q}")
                nc.gpsimd.indirect_dma_start(
                    out=v[:],
                    out_offset=None,
                    in_=fbt[:],
                    in_offset=bass.IndirectOffsetOnAxis(ap=offs[q][:, t:t + 1],
                                                        axis=0),
                )
                vt.append(v)
            # per-partition bilinear weights ride ScalarE's activation scale
            sc = [gpool.tile([128, C], f32, tag=f"sc{q}", name=f"sc{q}")
                  for q in range(4)]
            for q in range(4):
                sx, sy = wpairs[q]
                wq = small.tile([128, 1], f32, tag=f"wq{q}")
                nc.vector.tensor_tensor(out=wq[:], in0=sx[:, t:t + 1],
                                        in1=sy[:, t:t + 1], op=Alu.mult)
                nc.scalar.activation(out=sc[q][:], in_=vt[q][:], func=Act.Copy,
                                     scale=wq[:])
            acc = gpool.tile([128, C], f32, tag="acc")
            tmp = gpool.tile([128, C], f32, tag="tmp")
            nc.vector.tensor_add(out=tmp[:], in0=sc[0][:], in1=sc[1][:])
            nc.vector.tensor_add(out=acc[:], in0=sc[2][:], in1=sc[3][:])
            nc.vector.tensor_add(out=acc[:], in0=acc[:], in1=tmp[:])

            anchors = (2 * t, 2 * t + 1) if t < 4 else (8,)
            for a in anchors:
                half = (a % 2) * 64
                for chh in range(2):
                    g = a * 2 + chh
                    bt_ps = ppool.tile([128, K], f32, tag="btps")
                    nc.tensor.transpose(
                        out=bt_ps[:],
                        in_=acc[half:half + 64, chh * 128:(chh + 1) * 128],
                        identity=rb_sb[half:half + 64,
                                       ID_OFF + half:ID_OFF + half + 64],
                    )
                    app(a16h_all[:, 64 * g:64 * g + 64],
                        at16_all[:, 64 * g:64 * g + 64],
                        bt_ps[:],
                        out1_sb[:, g], out116_sb[:, g], out1h_sb[:, g])
                    # stagger: emit app2 one group late so independent app1
                    # work separates the dependent app1(g)->app2(g) stages
                    if pend_g is not None:
                        pg = pend_g
                        o2 = small.tile([128, K], f32, tag="o2")
                        app(out1h_sb[:, pg], out116_sb[:, pg],
                            rb_sb[:, CT_OFF + 64 * pg:CT_OFF + 64 * pg + 64],
                            o2[:], None, None)
                        nc.sync.dma_start(out=out_v[:, pg], in_=o2[:])
                    pend_g = g

        o2 = small.tile([128, K], f32, tag="o2", name="o2_last")
        app(out1h_sb[:, pend_g], out116_sb[:, pend_g],
            rb_sb[:, CT_OFF + 64 * pend_g:CT_OFF + 64 * pend_g + 64],
            o2[:], None, None)
        nc.sync.dma_start(out=out_v[:, pend_g], in_=o2[:])

        for p in (cpsum, ppool, small, ecpool, relpool, gpool, singles):
            p.release()

    if not nc.is_finalized():
        nc.finalize()
    return nc


def _host_prep(inputs):
    """Per-core input maps from the full inputs (pure layout transforms)."""
    import ml_dtypes

    ra = np.asarray(inputs["rois_feature_a"], dtype=np.float32).reshape(A, N, K, C)
    rc = np.asarray(inputs["rois_feature_c"], dtype=np.float32).reshape(A, N, K, C)
    fbf = np.asarray(inputs["feature_b"], dtype=np.float32)
    wr = np.asarray(inputs["W_reg"], dtype=np.float32)
    br = np.asarray(inputs["b_reg"], dtype=np.float32)

    # conv weights: [A, C, dy, dx] -> [c_lo, (c_hi dy dx), a] flat [128, 1152]
    w = wr.transpose(1, 2, 3, 0).reshape(2, 128, BS, BS, A)
    w = w.transpose(1, 0, 2, 3, 4).reshape(128, 128 * A)

    r = (0.5 * (BS - 1) + BS * np.arange(F)).astype(np.float32)
    xc_g = np.broadcast_to(r[None, :], (F, F))
    yc_g = np.ascontiguousarray(xc_g.T)
    pad = ST * 128 - NS
    xc_s = np.concatenate([np.broadcast_to(xc_g.reshape(1, K), (A, K)).reshape(NS),
                           np.full(pad, 31.5, np.float32)]).astype(np.float32)
    yc_s = np.concatenate([np.broadcast_to(yc_g.reshape(1, K), (A, K)).reshape(NS),
                           np.full(pad, 31.5, np.float32)]).astype(np.float32)

    def to_pt(v):  # [640] -> [128, 5]
        return np.ascontiguousarray(v.reshape(ST, 128).T)

    in_maps = []
    for n in range(N):
        fbw16 = np.zeros((128, NFB16E), ml_dtypes.bfloat16)
        fbw16[:, W_OFF:W_OFF + 1152] = w.astype(ml_dtypes.bfloat16)
        fbw16[0, B_OFF:B_OFF + A] = br.astype(ml_dtypes.bfloat16)
        fbw16[0, ONE_OFF:ONE_OFF + K] = 1.0
        fb_conv = fbf[n].reshape(C, F, BS, F, BS).transpose(0, 2, 4, 1, 3)
        fbw16[:, FB_OFF:] = (fb_conv.reshape(2, 128, 8192 // 2)
                             .transpose(1, 0, 2).reshape(128, 8192)
                             .astype(ml_dtypes.bfloat16))
        fbw_h = np.frombuffer(np.ascontiguousarray(fbw16).tobytes(),
                              dtype=np.float32).reshape(128, NFBW)

        a_t = ra[:, n].transpose(0, 2, 1).reshape(GT, 128, K)   # [(a c) k]
        c_t = rc[:, n].transpose(0, 2, 1).reshape(GT, 128, K)
        at_rows = np.ascontiguousarray(a_t.transpose(1, 0, 2).reshape(128, 1152))
        ct_rows = np.ascontiguousarray(c_t.transpose(1, 0, 2).reshape(128, 1152))
        a16_pack = np.frombuffer(at_rows.astype(ml_dtypes.bfloat16).tobytes(),
                                 dtype=np.float32).reshape(128, 576)
        a16h_pack = np.frombuffer(at_rows.astype(np.float16).tobytes(),
                                  dtype=np.float32).reshape(128, 576)

        rb_h = np.zeros((128, NRB), np.float32)
        rb_h[:, CT_OFF:CT_OFF + 1152] = ct_rows
        rb_h[:, A16_OFF:A16_OFF + 576] = a16_pack
        rb_h[:, A16H_OFF:A16H_OFF + 576] = a16h_pack
        rb_h[:, XC_OFF:XC_OFF + ST] = to_pt(xc_s)
        rb_h[:, YC_OFF:YC_OFF + ST] = to_pt(yc_s)
        rb_h[:, ID_OFF:ID_OFF + 128] = np.eye(128, dtype=np.float32)

        fbt_n = np.ascontiguousarray(fbf[n].reshape(C, H * W).T)
        in_maps.append({"fbw": fbw_h, "rb": rb_h, "fbt": fbt_n})
    return in_maps


def _assemble(results):
    """Per-core 'out' [G, K] -> full [M, C, 1, 1]."""
    outs = []
    for n in range(N):
        o = np.asarray(results[n]["out"], dtype=np.float32).reshape(A, C, K)
        outs.append(o.transpose(0, 2, 1))            # [A, K, C]
    stk = np.stack(outs, axis=1)                      # [A, N, K, C]
    return np.ascontiguousarray(stk.reshape(M, C, 1, 1))


def kernel(**inputs):
    from concourse.bass_utils import run_bass_kernel_spmd

    if "nc" not in _CACHE:
        _CACHE["nc"] = _build_nc()
    nc = _CACHE["nc"]
    in_maps = _host_prep(inputs)
    res = run_bass_kernel_spmd(nc, in_maps, core_ids=list(range(N)))
    return _assemble(res.results)



# revision 4
# speedup vs baseline: 1.0319x; 1.0319x over previous
"""AnchorToAnchor fused kernel for 8 TRN2 NeuronCores.

Shards data-parallel over the batch axis N=8 (one batch element per core).
Per core the device graph computes:
  1. block-strided conv (BoxRegress) as 129 accumulated TensorE matmuls
     (bias folded in as a rank-1 update)
  2. tanh-regressed sample centers + bilinear gather offsets/weights
  3. bilinear sampling via indirect DMA gathers from the (host-transposed)
     feature map, combined with per-partition-scalar DVE ops
  4. two anchor-to-anchor relation (softmax attention) passes with groups
     (anchor, channel) on partitions and the K x K score matrix in the free
     dimension. ScalarE expands b per-j into fp16 so the DVE outer-product
     TT runs at its 2x perf mode; exp on ScalarE (fp16 in -> bf16 out);
     e*a multiply + 3 bf16 tree-add halvings + a short tensor_reduce give
     den/num (tensor_reduce has no DVE fast mode, tree-adds do); final
     num/den combine in fp32 with a fast approximate reciprocal.

Engine notes baked into this design (measured on HW): DVE is the bottleneck
(~0.96 GHz, fp32 TT 1x, 16-bit TT 2x, single-src up to 4x; broadcast APs
with a step-0 innermost dim force 1x); GPSIMD shares SBUF ports with DVE so
offloading bulk elementwise work there is a wash; ScalarE runs ~1 elem/cyc
at 1.2 GHz for any dtype and has its own port budget, so it carries the
broadcast-expands, exps and psum copies. Compute instructions can embed only
one semaphore wait; building with bacc.Bacc legalizes multi-wait cases via
event-semaphore instructions.

The host wrapper only reshapes/transposes inputs into device-friendly
layouts (pure permutations), runs the SPMD NEFF on cores 0-7, and
re-assembles the full output.
"""

import sys

for _p in ("/opt/trn_rl_repo",):
    if _p not in sys.path:
        sys.path.insert(0, _p)

import numpy as np

# Problem constants (hardcoded per the task spec).
N, C, H, W = 8, 256, 64, 64
A, BS = 9, 8
F = H // BS          # 8
K = F * F            # 64
M = A * N * K        # 4608
ALPHA = 0.1
G = A * C            # 2304 groups per core
GT = G // 128        # 18 group tiles
ST = 5               # sample tiles of 128 (576 samples -> 4.5, padded)
NS = A * K           # 576 samples per core

# fbw16 blob: bf16 element offsets (stored as f32 words, bitcast on device)
W_OFF = 0            # conv weights [128, 128*9] bf16
B_OFF = 1152         # bias row (row 0 only) [9] bf16
ONE_OFF = 1161       # ones row (row 0 only) [64] bf16
FB_OFF = 1226        # conv feature [128, 8192] bf16 (even offset)
NFB16E = FB_OFF + 8192   # 9418 bf16 elements
NFBW = NFB16E // 2       # 4709 f32 words

# rb blob column offsets (f32 words)
CT_OFF = 0           # c-tensor [128, 18*64] f32
A16_OFF = 1152       # bf16 a-tensor packed [128, 576]
A16H_OFF = 1728      # fp16 a-tensor packed [128, 576]
XC_OFF = 2304        # x centers [128, 5]
YC_OFF = 2309        # y centers [128, 5]
ID_OFF = 2314        # identity [128, 128] f32
NRB = 2442

_CACHE = {}


def _build_nc():
    import concourse.bass as bass
    import concourse.bacc as bacc
    import concourse.tile as tile
    from concourse import mybir

    f32 = mybir.dt.float32
    bf16 = mybir.dt.bfloat16
    f16 = mybir.dt.float16
    i32 = mybir.dt.int32
    Alu = mybir.AluOpType
    Act = mybir.ActivationFunctionType

    nc = bacc.Bacc(None)

    fbw = nc.declare_dram_parameter("fbw", [128, NFBW], f32, isOutput=False)
    rb = nc.declare_dram_parameter("rb", [128, NRB], f32, isOutput=False)
    fbt = nc.declare_dram_parameter("fbt", [H * W, C], f32, isOutput=False)
    out_d = nc.declare_dram_parameter("out", [G, K], f32, isOutput=True)

    with tile.TileContext(nc) as tc:
        singles = tc.alloc_tile_pool(name="singles", bufs=1)
        gpool = tc.alloc_tile_pool(name="gpool", bufs=3)
        relpool = tc.alloc_tile_pool(name="relpool", bufs=4)
        ecpool = tc.alloc_tile_pool(name="ecpool", bufs=2)
        small = tc.alloc_tile_pool(name="small", bufs=4)
        ppool = tc.alloc_tile_pool(name="ppool", bufs=2, space="PSUM")
        cpsum = tc.alloc_tile_pool(name="cpsum", bufs=1, space="PSUM")

        # ---- resident loads (two blobs; fbw split over 4 queues) -----------
        fbw_sb = singles.tile([128, NFBW], f32)
        splits = [0, FB_OFF // 2, FB_OFF // 2 + 1024, FB_OFF // 2 + 2048,
                  FB_OFF // 2 + 3072, NFBW]
        for q in range(5):
            nc.sync.dma_start(out=fbw_sb[:, splits[q]:splits[q + 1]],
                              in_=fbw[:, splits[q]:splits[q + 1]])
        rb_sb = singles.tile([128, NRB], f32)
        nc.sync.dma_start(out=rb_sb[:, :NRB // 2], in_=rb[:, :NRB // 2])
        nc.sync.dma_start(out=rb_sb[:, NRB // 2:], in_=rb[:, NRB // 2:])

        # DVE pre-touch of the rb blob: its single DMA wait lands here so
        # later DVE consumers of rb carry no fresh semaphore.
        dve_touch = singles.tile([128, 1], f32)
        nc.vector.tensor_copy(out=dve_touch[:], in_=rb_sb[:, 0:1])

        fbw16 = fbw_sb[:].bitcast(bf16)                           # [128, 9418]
        at16_all = rb_sb[:, A16_OFF:A16_OFF + 576].bitcast(bf16)  # [128, 1152]
        a16h_all = rb_sb[:, A16H_OFF:A16H_OFF + 576].bitcast(f16)  # [128, 1152]
        ident = rb_sb[:, ID_OFF:ID_OFF + 128]
        xc_t = rb_sb[:, XC_OFF:XC_OFF + ST]
        yc_t = rb_sb[:, YC_OFF:YC_OFF + ST]

        # ---- conv (BoxRegress) in bf16 (4x PE rate), out [a, ij] -----------
        conv_ps = cpsum.tile([A, K], f32)
        for k in range(128):
            nc.tensor.matmul(
                out=conv_ps[:],
                lhsT=fbw16[:, W_OFF + 9 * k:W_OFF + 9 * k + 9],
                rhs=fbw16[:, FB_OFF + 64 * k:FB_OFF + 64 * k + 64],
                start=(k == 0),
                stop=False,
            )
        nc.tensor.matmul(
            out=conv_ps[:],
            lhsT=fbw16[0:1, B_OFF:B_OFF + A],
            rhs=fbw16[0:1, ONE_OFF:ONE_OFF + K],
            start=False,
            stop=True,
        )
        conv_s = singles.tile([A, K], f32)
        nc.scalar.copy(out=conv_s[:], in_=conv_ps[:])

        # reorg [a, ij] -> regs[(a ij) % 128, (a ij) // 128]
        regs = singles.tile([128, ST], f32)
        nc.scalar.memzero(regs[:])
        for t in range(ST):
            a0 = 2 * t
            nparts = 2 if t < 4 else 1
            nc.sync.dma_start(
                out=regs[0:64 * nparts, t:t + 1],
                in_=conv_s[a0:a0 + nparts, :],
            )

        # ---- centers, offsets, weights -------------------------------------
        th = small.tile([128, ST], f32)
        for t in range(ST):
            nc.scalar.activation(out=th[:, t:t + 1], in_=regs[:, t:t + 1],
                                 func=Act.Tanh)
        t8 = small.tile([128, ST], f32)
        nc.vector.tensor_scalar_mul(t8[:], th[:], ALPHA * BS)
        px = small.tile([128, ST], f32)
        py = small.tile([128, ST], f32)
        nc.vector.tensor_add(out=px[:], in0=t8[:], in1=xc_t)
        nc.vector.tensor_add(out=py[:], in0=t8[:], in1=yc_t)

        def floor_of(src, dst_f):
            ri = small.tile([128, ST], i32, tag="fl_i")
            nc.vector.tensor_copy(out=ri[:], in_=src[:])
            rf = small.tile([128, ST], f32, tag="fl_f")
            nc.vector.tensor_copy(out=rf[:], in_=ri[:])
            gt = small.tile([128, ST], f32, tag="fl_g")
            nc.vector.tensor_tensor(out=gt[:], in0=rf[:], in1=src[:],
                                    op=Alu.is_gt)
            nc.vector.tensor_sub(out=dst_f[:], in0=rf[:], in1=gt[:])

        x0f = small.tile([128, ST], f32)
        y0f = small.tile([128, ST], f32)
        floor_of(px, x0f)
        floor_of(py, y0f)
        wx = small.tile([128, ST], f32)
        wy = small.tile([128, ST], f32)
        nc.vector.tensor_sub(out=wx[:], in0=px[:], in1=x0f[:])
        nc.vector.tensor_sub(out=wy[:], in0=py[:], in1=y0f[:])
        ux = small.tile([128, ST], f32)
        uy = small.tile([128, ST], f32)
        nc.vector.tensor_scalar(out=ux[:], in0=wx[:], scalar1=-1.0, scalar2=1.0,
                                op0=Alu.mult, op1=Alu.add)
        nc.vector.tensor_scalar(out=uy[:], in0=wy[:], scalar1=-1.0, scalar2=1.0,
                                op0=Alu.mult, op1=Alu.add)

        o00f = small.tile([128, ST], f32)
        nc.vector.tensor_scalar(out=o00f[:], in0=y0f[:], scalar1=float(W),
                                scalar2=None, op0=Alu.mult)
        nc.vector.tensor_add(out=o00f[:], in0=o00f[:], in1=x0f[:])
        offs = []
        for d in (0.0, 1.0, 64.0, 65.0):
            of = small.tile([128, ST], f32, tag="of_f")
            if d == 0.0:
                nc.vector.tensor_copy(out=of[:], in_=o00f[:])
            else:
                nc.vector.tensor_scalar_add(of[:], o00f[:], d)
            oi = small.tile([128, ST], i32, tag=f"of_i{d}")
            nc.vector.tensor_copy(out=oi[:], in_=of[:])
            offs.append(oi)

        # ---- per sample-tile gather + bilinear; per anchor transpose + apps -
        wpairs = [(ux, uy), (wx, uy), (ux, wy), (wx, wy)]
        out1_sb = singles.tile([128, GT, K], f32)
        out116_sb = singles.tile([128, GT, K], bf16)
        out1h_sb = singles.tile([128, GT, K], f16)
        pend_g = None
        out_v = out_d.rearrange("(g p) k -> p g k", p=128)

        def app(a_h, a_b, b_f, o_f, o_b, o_h):
            # ScalarE materializes the per-j broadcast of b in fp16 so the
            # DVE outer-product TT has step-1 fp16 operands and runs at 2x.
            bexp = relpool.tile([128, K, K], f16, tag="bexp")
            nc.scalar.activation(out=bexp[:],
                                 in_=b_f.unsqueeze(2).to_broadcast([128, K, K]),
                                 func=Act.Copy)
            rel = relpool.tile([128, K, K], f16, tag="rel")
            nc.vector.tensor_tensor(
                out=rel[:],
                in0=a_h.unsqueeze(1).to_broadcast([128, K, K]),
                in1=bexp[:],
                op=Alu.mult,
            )
            ec = ecpool.tile([128, 2, K, K], bf16, tag="ec")
            nc.scalar.activation(out=ec[:, 0], in_=rel[:], func=Act.Exp)
            nc.vector.tensor_tensor(
                out=ec[:, 1],
                in0=ec[:, 0],
                in1=a_b.unsqueeze(1).to_broadcast([128, K, K]),
                op=Alu.mult,
            )
            # bf16 tree-adds run at 2x on DVE while tensor_reduce is stuck at
            # 1x; halving all the way to width 1 beats tree+reduce hybrids.
            t0 = ecpool.tile([128, 2, K, 32], bf16, tag="t0")
            nc.vector.tensor_tensor(out=t0[:], in0=ec[:, :, :, 0:32],
                                    in1=ec[:, :, :, 32:64], op=Alu.add)
            t1 = ecpool.tile([128, 2, K, 16], bf16, tag="t1")
            nc.vector.tensor_tensor(out=t1[:], in0=t0[:, :, :, 0:16],
                                    in1=t0[:, :, :, 16:32], op=Alu.add)
            t2 = ecpool.tile([128, 2, K, 8], bf16, tag="t2")
            nc.vector.tensor_tensor(out=t2[:], in0=t1[:, :, :, 0:8],
                                    in1=t1[:, :, :, 8:16], op=Alu.add)
            t3 = ecpool.tile([128, 2, K, 4], bf16, tag="t3")
            nc.vector.tensor_tensor(out=t3[:], in0=t2[:, :, :, 0:4],
                                    in1=t2[:, :, :, 4:8], op=Alu.add)
            t4 = ecpool.tile([128, 2, K, 2], bf16, tag="t4")
            nc.vector.tensor_tensor(out=t4[:], in0=t3[:, :, :, 0:2],
                                    in1=t3[:, :, :, 2:4], op=Alu.add)
            dn = small.tile([128, 2, K], f32, tag="dn")
            nc.vector.tensor_tensor(out=dn[:], in0=t4[:, :, :, 0],
                                    in1=t4[:, :, :, 1], op=Alu.add)
            inv = small.tile([128, K], f32, tag="inv")
            nc.vector.reciprocal_approx_fast(out=inv[:], in_=dn[:, 0])
            r = small.tile([128, K], f32, tag="r")
            nc.vector.tensor_mul(out=r[:], in0=dn[:, 1], in1=inv[:])
            nc.vector.tensor_add(out=o_f[:], in0=r[:], in1=b_f[:])
            if o_b is not None:
                nc.scalar.copy(out=o_b[:], in_=o_f[:])
                # fp16 copy on DVE: app2's rel follows in-engine order, no
                # ScalarE round-trip on the serial app1->app2 chain
                nc.vector.tensor_copy(out=o_h[:], in_=o_f[:])

        for t in range(ST):
            vt = []
            for q in range(4):
                v = gpool.tile([128, C], f32, tag=f"v{q}")
                nc.gpsimd.indirect_dma_start(
                    out=v[:],
                    out_offset=None,
                    in_=fbt[:],
                    in_offset=bass.IndirectOffsetOnAxis(ap=offs[q][:, t:t + 1],
                                                        axis=0),
                )
                vt.append(v)
            # per-partition bilinear weights ride ScalarE's activation scale
            sc = [gpool.tile([128, C], f32, tag=f"sc{q}", name=f"sc{q}")
                  for q in range(4)]
            for q in range(4):
                sx, sy = wpairs[q]
                wq = small.tile([128, 1], f32, tag=f"wq{q}")
                nc.vector.tensor_tensor(out=wq[:], in0=sx[:, t:t + 1],
                                        in1=sy[:, t:t + 1], op=Alu.mult)
                nc.scalar.activation(out=sc[q][:], in_=vt[q][:], func=Act.Copy,
                                     scale=wq[:])
            acc = gpool.tile([128, C], f32, tag="acc")
            tmp = gpool.tile([128, C], f32, tag="tmp")
            nc.vector.tensor_add(out=tmp[:], in0=sc[0][:], in1=sc[1][:])
            nc.vector.tensor_add(out=acc[:], in0=sc[2][:], in1=sc[3][:])
            nc.vector.tensor_add(out=acc[:], in0=acc[:], in1=tmp[:])

            anchors = (2 * t, 2 * t + 1) if t < 4 else (8,)
            for a in anchors:
                half = (a % 2) * 64
                for chh in range(2):
                    g = a * 2 + chh
                    bt_ps = ppool.tile([128, K], f32, tag="btps")
                    nc.tensor.transpose(
                        out=bt_ps[:],
                        in_=acc[half:half + 64, chh * 128:(chh + 1) * 128],
                        identity=rb_sb[half:half + 64,
                                       ID_OFF + half:ID_OFF + half + 64],
                    )
                    app(a16h_all[:, 64 * g:64 * g + 64],
                        at16_all[:, 64 * g:64 * g + 64],
                        bt_ps[:],
                        out1_sb[:, g], out116_sb[:, g], out1h_sb[:, g])
                    # stagger: emit app2 one group late so independent app1
                    # work separates the dependent app1(g)->app2(g) stages
                    if pend_g is not None:
                        pg = pend_g
                        o2 = small.tile([128, K], f32, tag="o2")
                        app(out1h_sb[:, pg], out116_sb[:, pg],
                            rb_sb[:, CT_OFF + 64 * pg:CT_OFF + 64 * pg + 64],
                            o2[:], None, None)
                        nc.sync.dma_start(out=out_v[:, pg], in_=o2[:])
                    pend_g = g

        o2 = small.tile([128, K], f32, tag="o2", name="o2_last")
        app(out1h_sb[:, pend_g], out116_sb[:, pend_g],
            rb_sb[:, CT_OFF + 64 * pend_g:CT_OFF + 64 * pend_g + 64],
            o2[:], None, None)
        nc.sync.dma_start(out=out_v[:, pend_g], in_=o2[:])

        for p in (cpsum, ppool, small, ecpool, relpool, gpool, singles):
            p.release()

    if not nc.is_finalized():
        nc.finalize()
    return nc


def _host_prep(inputs):
    """Per-core input maps from the full inputs (pure layout transforms)."""
    import ml_dtypes

    ra = np.asarray(inputs["rois_feature_a"], dtype=np.float32).reshape(A, N, K, C)
    rc = np.asarray(inputs["rois_feature_c"], dtype=np.float32).reshape(A, N, K, C)
    fbf = np.asarray(inputs["feature_b"], dtype=np.float32)
    wr = np.asarray(inputs["W_reg"], dtype=np.float32)
    br = np.asarray(inputs["b_reg"], dtype=np.float32)

    # conv weights: [A, C, dy, dx] -> [c_lo, (c_hi dy dx), a] flat [128, 1152]
    w = wr.transpose(1, 2, 3, 0).reshape(2, 128, BS, BS, A)
    w = w.transpose(1, 0, 2, 3, 4).reshape(128, 128 * A)

    r = (0.5 * (BS - 1) + BS * np.arange(F)).astype(np.float32)
    xc_g = np.broadcast_to(r[None, :], (F, F))
    yc_g = np.ascontiguousarray(xc_g.T)
    pad = ST * 128 - NS
    xc_s = np.concatenate([np.broadcast_to(xc_g.reshape(1, K), (A, K)).reshape(NS),
                           np.full(pad, 31.5, np.float32)]).astype(np.float32)
    yc_s = np.concatenate([np.broadcast_to(yc_g.reshape(1, K), (A, K)).reshape(NS),
                           np.full(pad, 31.5, np.float32)]).astype(np.float32)

    def to_pt(v):  # [640] -> [128, 5]
        return np.ascontiguousarray(v.reshape(ST, 128).T)

    in_maps = []
    for n in range(N):
        fbw16 = np.zeros((128, NFB16E), ml_dtypes.bfloat16)
        fbw16[:, W_OFF:W_OFF + 1152] = w.astype(ml_dtypes.bfloat16)
        fbw16[0, B_OFF:B_OFF + A] = br.astype(ml_dtypes.bfloat16)
        fbw16[0, ONE_OFF:ONE_OFF + K] = 1.0
        fb_conv = fbf[n].reshape(C, F, BS, F, BS).transpose(0, 2, 4, 1, 3)
        fbw16[:, FB_OFF:] = (fb_conv.reshape(2, 128, 8192 // 2)
                             .transpose(1, 0, 2).reshape(128, 8192)
                             .astype(ml_dtypes.bfloat16))
        fbw_h = np.frombuffer(np.ascontiguousarray(fbw16).tobytes(),
                              dtype=np.float32).reshape(128, NFBW)

        a_t = ra[:, n].transpose(0, 2, 1).reshape(GT, 128, K)   # [(a c) k]
        c_t = rc[:, n].transpose(0, 2, 1).reshape(GT, 128, K)
        at_rows = np.ascontiguousarray(a_t.transpose(1, 0, 2).reshape(128, 1152))
        ct_rows = np.ascontiguousarray(c_t.transpose(1, 0, 2).reshape(128, 1152))
        a16_pack = np.frombuffer(at_rows.astype(ml_dtypes.bfloat16).tobytes(),
                                 dtype=np.float32).reshape(128, 576)
        a16h_pack = np.frombuffer(at_rows.astype(np.float16).tobytes(),
                                  dtype=np.float32).reshape(128, 576)

        rb_h = np.zeros((128, NRB), np.float32)
        rb_h[:, CT_OFF:CT_OFF + 1152] = ct_rows
        rb_h[:, A16_OFF:A16_OFF + 576] = a16_pack
        rb_h[:, A16H_OFF:A16H_OFF + 576] = a16h_pack
        rb_h[:, XC_OFF:XC_OFF + ST] = to_pt(xc_s)
        rb_h[:, YC_OFF:YC_OFF + ST] = to_pt(yc_s)
        rb_h[:, ID_OFF:ID_OFF + 128] = np.eye(128, dtype=np.float32)

        fbt_n = np.ascontiguousarray(fbf[n].reshape(C, H * W).T)
        in_maps.append({"fbw": fbw_h, "rb": rb_h, "fbt": fbt_n})
    return in_maps


def _assemble(results):
    """Per-core 'out' [G, K] -> full [M, C, 1, 1]."""
    outs = []
    for n in range(N):
        o = np.asarray(results[n]["out"], dtype=np.float32).reshape(A, C, K)
        outs.append(o.transpose(0, 2, 1))            # [A, K, C]
    stk = np.stack(outs, axis=1)                      # [A, N, K, C]
    return np.ascontiguousarray(stk.reshape(M, C, 1, 1))


def kernel(**inputs):
    from concourse.bass_utils import run_bass_kernel_spmd

    if "nc" not in _CACHE:
        _CACHE["nc"] = _build_nc()
    nc = _CACHE["nc"]
    in_maps = _host_prep(inputs)
    res = run_bass_kernel_spmd(nc, in_maps, core_ids=list(range(N)))
    return _assemble(res.results)



# revision 9
# speedup vs baseline: 1.0429x; 1.0106x over previous
"""AnchorToAnchor fused kernel for 8 TRN2 NeuronCores.

Shards data-parallel over the batch axis N=8 (one batch element per core).
Per core the device graph computes:
  1. block-strided conv (BoxRegress) as 129 accumulated TensorE matmuls
     (bias folded in as a rank-1 update)
  2. tanh-regressed sample centers + bilinear gather offsets/weights
  3. bilinear sampling via indirect DMA gathers from the (host-transposed)
     feature map, combined with per-partition-scalar DVE ops
  4. two anchor-to-anchor relation (softmax attention) passes with groups
     (anchor, channel) on partitions and the K x K score matrix in the free
     dimension. ScalarE expands b per-j into fp16 so the DVE outer-product
     TT runs at its 2x perf mode; exp on ScalarE (fp16 in -> bf16 out);
     e*a multiply + 3 bf16 tree-add halvings + a short tensor_reduce give
     den/num (tensor_reduce has no DVE fast mode, tree-adds do); final
     num/den combine in fp32 with a fast approximate reciprocal.

Engine notes baked into this design (measured on HW): DVE is the bottleneck
(~0.96 GHz, fp32 TT 1x, 16-bit TT 2x, single-src up to 4x; broadcast APs
with a step-0 innermost dim force 1x); GPSIMD shares SBUF ports with DVE so
offloading bulk elementwise work there is a wash; ScalarE runs ~1 elem/cyc
at 1.2 GHz for any dtype and has its own port budget, so it carries the
broadcast-expands, exps and psum copies. Compute instructions can embed only
one semaphore wait; building with bacc.Bacc legalizes multi-wait cases via
event-semaphore instructions.

The host wrapper only reshapes/transposes inputs into device-friendly
layouts (pure permutations), runs the SPMD NEFF on cores 0-7, and
re-assembles the full output.
"""

import sys

for _p in ("/opt/trn_rl_repo",):
    if _p not in sys.path:
        sys.path.insert(0, _p)

import numpy as np

# Problem constants (hardcoded per the task spec).
N, C, H, W = 8, 256, 64, 64
A, BS = 9, 8
F = H // BS          # 8
K = F * F            # 64
M = A * N * K        # 4608
ALPHA = 0.1
G = A * C            # 2304 groups per core
GT = G // 128        # 18 group tiles
ST = 5               # sample tiles of 128 (576 samples -> 4.5, padded)
NS = A * K           # 576 samples per core

# fbw16 blob: bf16 element offsets (stored as f32 words, bitcast on device)
W_OFF = 0            # conv weights [128, 128*9] bf16
B_OFF = 1152         # bias row (row 0 only) [9] bf16
ONE_OFF = 1161       # ones row (row 0 only) [64] bf16
FB_OFF = 1226        # conv feature [128, 8192] bf16 (even offset)
NFB16E = FB_OFF + 8192   # 9418 bf16 elements
NFBW = NFB16E // 2       # 4709 f32 words
WSPLIT = FB_OFF // 2     # 613 f32 words: weights+bias+ones chunk

# rb blob column offsets (f32 words)
CT_OFF = 0           # c-tensor [128, 18*64] f32
A16_OFF = 1152       # bf16 a-tensor packed [128, 576]
A16H_OFF = 1728      # fp16 a-tensor packed [128, 576]
XC_OFF = 2304        # x centers [128, 5]
YC_OFF = 2309        # y centers [128, 5]
ID_OFF = 2314        # identity [128, 128] f32
SE_OFF = 2442        # even-anchor selector [9, 10] f32
SO_OFF = 2452        # odd-anchor selector [9, 10] f32
ID16_OFF = 2462      # bf16 64-block identity [128, 64] bf16 (32 words)
NRB = 2494

_CACHE = {}


def _build_nc():
    import concourse.bass as bass
    import concourse.bacc as bacc
    import concourse.tile as tile
    from concourse import mybir

    f32 = mybir.dt.float32
    bf16 = mybir.dt.bfloat16
    f16 = mybir.dt.float16
    i32 = mybir.dt.int32
    Alu = mybir.AluOpType
    Act = mybir.ActivationFunctionType

    nc = bacc.Bacc(None)

    fbw = nc.declare_dram_parameter("fbw", [128, NFBW], f32, isOutput=False)
    rb = nc.declare_dram_parameter("rb", [128, NRB], f32, isOutput=False)
    fbt2 = nc.declare_dram_parameter("fbt2", [H * W - 1, 2 * C], bf16,
                                     isOutput=False)
    out_d = nc.declare_dram_parameter("out", [G, K], f32, isOutput=True)

    with tile.TileContext(nc) as tc:
        singles = tc.alloc_tile_pool(name="singles", bufs=1)
        gpool = tc.alloc_tile_pool(name="gpool", bufs=3)
        relpool = tc.alloc_tile_pool(name="relpool", bufs=4)
        ecpool = tc.alloc_tile_pool(name="ecpool", bufs=2)
        small = tc.alloc_tile_pool(name="small", bufs=4)
        ppool = tc.alloc_tile_pool(name="ppool", bufs=2, space="PSUM")
        cpsum = tc.alloc_tile_pool(name="cpsum", bufs=1, space="PSUM")

        # ---- resident loads (fbw in 5 dep-separated tiles; fbt2 split) -----
        w_sb = singles.tile([128, WSPLIT], f32)
        nc.sync.dma_start(out=w_sb[:], in_=fbw[:, 0:WSPLIT])
        fb_sb = []
        for c in range(4):
            fbc = singles.tile([128, 1024], f32, name=f"fbc{c}")
            nc.sync.dma_start(out=fbc[:],
                              in_=fbw[:, WSPLIT + 1024 * c:WSPLIT + 1024 * (c + 1)])
            fb_sb.append(fbc)
        rb_sb = singles.tile([128, NRB], f32)
        nc.sync.dma_start(out=rb_sb[:, :NRB // 2], in_=rb[:, :NRB // 2])
        nc.sync.dma_start(out=rb_sb[:, NRB // 2:], in_=rb[:, NRB // 2:])

        # DVE pre-touch of the rb blob: its single DMA wait lands here so
        # later DVE consumers of rb carry no fresh semaphore.
        dve_touch = singles.tile([128, 1], f32)
        nc.vector.tensor_copy(out=dve_touch[:], in_=rb_sb[:, 0:1])

        w16 = w_sb[:].bitcast(bf16)                               # [128, 1226]
        fb16 = [t[:].bitcast(bf16) for t in fb_sb]                # [128, 2048]
        at16_all = rb_sb[:, A16_OFF:A16_OFF + 576].bitcast(bf16)  # [128, 1152]
        a16h_all = rb_sb[:, A16H_OFF:A16H_OFF + 576].bitcast(f16)  # [128, 1152]
        id16 = rb_sb[:, ID16_OFF:ID16_OFF + 32].bitcast(bf16)     # [128, 64]
        xcyc = rb_sb[:, XC_OFF:XC_OFF + 2 * ST]                   # [128, 10]

        # ---- HAM warm-up: ~3.4us of dep-free dummy matmuls so the conv and
        # the first transposes run at 2.4 GHz instead of the cold 1.2 --------
        dummy = singles.tile([128, 256], f32)
        nc.vector.memset(dummy[:], 0.0)
        dummy16 = dummy[:].bitcast(bf16)
        warm_ps = cpsum.tile([128, 512], f32)
        for i in range(8):
            nc.tensor.matmul(out=warm_ps[:], lhsT=dummy16[:, 0:128],
                             rhs=dummy16[:, 0:512], start=(i == 0),
                             stop=(i == 7))

        # ---- conv (BoxRegress) in bf16, out [a, ij]; per-chunk DMA deps ----
        conv_ps = cpsum.tile([A, K], f32)
        for k in range(128):
            c, kk = divmod(k, 32)
            nc.tensor.matmul(
                out=conv_ps[:],
                lhsT=w16[:, W_OFF + 9 * k:W_OFF + 9 * k + 9],
                rhs=fb16[c][:, 64 * kk:64 * kk + 64],
                start=(k == 0),
                stop=False,
            )
        nc.tensor.matmul(
            out=conv_ps[:],
            lhsT=w16[0:1, B_OFF:B_OFF + A],
            rhs=w16[0:1, ONE_OFF:ONE_OFF + K],
            start=False,
            stop=True,
        )
        conv_s = singles.tile([A, K], f32)
        nc.scalar.copy(out=conv_s[:], in_=conv_ps[:])

        # reorg [a, ij] -> regs10[(a ij) % 128, dup((a ij) // 128)] via PE
        # selector matmuls (no tiny DMAs); columns 0-4 == 5-9 so the x/y
        # center chains below run joint on [128, 10] tiles.
        regs_ps = cpsum.tile([128, 2 * ST], f32)
        nc.tensor.matmul(out=regs_ps[0:64, :], lhsT=conv_s[:],
                         rhs=rb_sb[0:A, SE_OFF:SE_OFF + 2 * ST],
                         start=True, stop=True)
        nc.tensor.matmul(out=regs_ps[64:128, :], lhsT=conv_s[:],
                         rhs=rb_sb[0:A, SO_OFF:SO_OFF + 2 * ST],
                         start=True, stop=True, tile_position=(0, 64))

        # ---- centers, offsets, weights (joint x/y on [128, 10]) ------------
        th10 = small.tile([128, 2 * ST], f32)
        nc.scalar.activation(out=th10[:], in_=regs_ps[:], func=Act.Tanh)
        t8 = small.tile([128, 2 * ST], f32)
        nc.vector.tensor_scalar_mul(t8[:], th10[:], ALPHA * BS)
        pxy = small.tile([128, 2 * ST], f32)
        nc.vector.tensor_add(out=pxy[:], in0=t8[:], in1=xcyc)

        ri = small.tile([128, 2 * ST], i32)
        nc.vector.tensor_copy(out=ri[:], in_=pxy[:])
        rf = small.tile([128, 2 * ST], f32)
        nc.vector.tensor_copy(out=rf[:], in_=ri[:])
        gt = small.tile([128, 2 * ST], f32)
        nc.vector.tensor_tensor(out=gt[:], in0=rf[:], in1=pxy[:], op=Alu.is_gt)
        xy0f = small.tile([128, 2 * ST], f32)
        nc.vector.tensor_sub(out=xy0f[:], in0=rf[:], in1=gt[:])
        wxy = small.tile([128, 2 * ST], f32)
        nc.vector.tensor_sub(out=wxy[:], in0=pxy[:], in1=xy0f[:])
        uxy = small.tile([128, 2 * ST], f32)
        nc.vector.tensor_scalar(out=uxy[:], in0=wxy[:], scalar1=-1.0,
                                scalar2=1.0, op0=Alu.mult, op1=Alu.add)
        wx, wy = wxy[:, 0:ST], wxy[:, ST:2 * ST]
        ux, uy = uxy[:, 0:ST], uxy[:, ST:2 * ST]

        o00f = small.tile([128, ST], f32)
        nc.vector.tensor_scalar(out=o00f[:], in0=xy0f[:, ST:2 * ST],
                                scalar1=float(W), scalar2=None, op0=Alu.mult)
        nc.vector.tensor_add(out=o00f[:], in0=o00f[:], in1=xy0f[:, 0:ST])
        oi0 = small.tile([128, ST], i32)
        nc.vector.tensor_copy(out=oi0[:], in_=o00f[:])
        o64f = small.tile([128, ST], f32)
        nc.vector.tensor_scalar_add(o64f[:], o00f[:], float(W))
        oi64 = small.tile([128, ST], i32)
        nc.vector.tensor_copy(out=oi64[:], in_=o64f[:])

        # bilinear weight products for all tiles at once: [128, 20] q-major
        wq20 = small.tile([128, 4 * ST], f32)
        nc.vector.tensor_tensor(out=wq20[:, 0:ST], in0=ux, in1=uy, op=Alu.mult)
        nc.vector.tensor_tensor(out=wq20[:, ST:2 * ST], in0=wx, in1=uy,
                                op=Alu.mult)
        nc.vector.tensor_tensor(out=wq20[:, 2 * ST:3 * ST], in0=ux, in1=wy,
                                op=Alu.mult)
        nc.vector.tensor_tensor(out=wq20[:, 3 * ST:4 * ST], in0=wx, in1=wy,
                                op=Alu.mult)

        # ---- per sample-tile gather + bilinear; per anchor transpose + apps -
        out1_sb = singles.tile([128, GT, K], f32)
        out116_sb = singles.tile([128, GT, K], bf16)
        out1h_sb = singles.tile([128, GT, K], f16)
        pend_g = None
        out_v = out_d.rearrange("(g p) k -> p g k", p=128)

        def app(a_h, a_b, b_f, o_f, o_b, o_h, halves=1):
            # ScalarE materializes the per-j broadcast of b in fp16 so the
            # DVE outer-product TT has step-1 fp16 operands and runs at 2x.
            # halves=2 splits along j to pipeline ScalarE/DVE stages when
            # this app is alone in flight (pipeline head / tail).
            bexp = relpool.tile([128, K, K], f16, tag="bexp")
            rel = relpool.tile([128, K, K], f16, tag="rel")
            ec = ecpool.tile([128, 2, K, K], bf16, tag="ec")
            t0 = ecpool.tile([128, 2, K, 32], bf16, tag="t0")
            t1 = ecpool.tile([128, 2, K, 16], bf16, tag="t1")
            t2 = ecpool.tile([128, 2, K, 8], bf16, tag="t2")
            t3 = ecpool.tile([128, 2, K, 4], bf16, tag="t3")
            t4 = ecpool.tile([128, 2, K, 2], bf16, tag="t4")
            dn = small.tile([128, 2, K], f32, tag="dn")
            inv = small.tile([128, K], f32, tag="inv")
            r = small.tile([128, K], f32, tag="r")
            jw = K // halves
            for h in range(halves):
                j0, j1 = h * jw, (h + 1) * jw
                nc.scalar.activation(
                    out=bexp[:, j0:j1],
                    in_=b_f[:, j0:j1].unsqueeze(2).to_broadcast([128, jw, K]),
                    func=Act.Copy)
                nc.vector.tensor_tensor(
                    out=rel[:, j0:j1],
                    in0=a_h.unsqueeze(1).to_broadcast([128, jw, K]),
                    in1=bexp[:, j0:j1],
                    op=Alu.mult,
                )
                nc.scalar.activation(out=ec[:, 0, j0:j1], in_=rel[:, j0:j1],
                                     func=Act.Exp)
                nc.vector.tensor_tensor(
                    out=ec[:, 1, j0:j1],
                    in0=ec[:, 0, j0:j1],
                    in1=a_b.unsqueeze(1).to_broadcast([128, jw, K]),
                    op=Alu.mult,
                )
                # bf16 tree-adds run at 2x on DVE while tensor_reduce is stuck
                # at 1x; halving all the way to width 1 wins.
                nc.vector.tensor_tensor(out=t0[:, :, j0:j1],
                                        in0=ec[:, :, j0:j1, 0:32],
                                        in1=ec[:, :, j0:j1, 32:64], op=Alu.add)
                nc.vector.tensor_tensor(out=t1[:, :, j0:j1],
                                        in0=t0[:, :, j0:j1, 0:16],
                                        in1=t0[:, :, j0:j1, 16:32], op=Alu.add)
                nc.vector.tensor_tensor(out=t2[:, :, j0:j1],
                                        in0=t1[:, :, j0:j1, 0:8],
                                        in1=t1[:, :, j0:j1, 8:16], op=Alu.add)
                nc.vector.tensor_tensor(out=t3[:, :, j0:j1],
                                        in0=t2[:, :, j0:j1, 0:4],
                                        in1=t2[:, :, j0:j1, 4:8], op=Alu.add)
                nc.vector.tensor_tensor(out=t4[:, :, j0:j1],
                                        in0=t3[:, :, j0:j1, 0:2],
                                        in1=t3[:, :, j0:j1, 2:4], op=Alu.add)
                nc.vector.tensor_tensor(out=dn[:, :, j0:j1],
                                        in0=t4[:, :, j0:j1, 0],
                                        in1=t4[:, :, j0:j1, 1], op=Alu.add)
                nc.vector.reciprocal_approx_fast(out=inv[:, j0:j1],
                                                 in_=dn[:, 0, j0:j1])
                nc.vector.tensor_mul(out=r[:, j0:j1], in0=dn[:, 1, j0:j1],
                                     in1=inv[:, j0:j1])
                nc.vector.tensor_add(out=o_f[:, j0:j1], in0=r[:, j0:j1],
                                     in1=b_f[:, j0:j1])
                if o_b is not None:
                    nc.scalar.copy(out=o_b[:, j0:j1], in_=o_f[:, j0:j1])
                    # fp16 copy on DVE: app2's rel follows in-engine order, no
                    # ScalarE round-trip on the serial app1->app2 chain
                    nc.vector.tensor_copy(out=o_h[:, j0:j1], in_=o_f[:, j0:j1])

        first_app = True
        for t in range(ST):
            # two row-pair gathers per tile: fbt2 row r = [pixel r | pixel
            # r+1] channels, so (y0,x0)+(y0,x1) come in one descriptor set.
            v0 = gpool.tile([128, 2 * C], bf16, tag="v0")
            nc.gpsimd.indirect_dma_start(
                out=v0[:], out_offset=None, in_=fbt2[:],
                in_offset=bass.IndirectOffsetOnAxis(ap=oi0[:, t:t + 1], axis=0),
            )
            v1 = gpool.tile([128, 2 * C], bf16, tag="v1")
            nc.gpsimd.indirect_dma_start(
                out=v1[:], out_offset=None, in_=fbt2[:],
                in_offset=bass.IndirectOffsetOnAxis(ap=oi64[:, t:t + 1], axis=0),
            )
            # per-partition bilinear weights ride ScalarE's activation scale
            sc = gpool.tile([128, 4, C], bf16, tag="sc")
            srcs = (v0[:, 0:C], v0[:, C:2 * C], v1[:, 0:C], v1[:, C:2 * C])
            for q in range(4):
                nc.scalar.activation(out=sc[:, q], in_=srcs[q], func=Act.Copy,
                                     scale=wq20[:, ST * q + t:ST * q + t + 1])
            acc = gpool.tile([128, C], bf16, tag="acc")
            tmp = gpool.tile([128, C], bf16, tag="tmp")
            nc.vector.tensor_add(out=tmp[:], in0=sc[:, 0], in1=sc[:, 1])
            nc.vector.tensor_add(out=acc[:], in0=sc[:, 2], in1=sc[:, 3])
            nc.vector.tensor_add(out=acc[:], in0=acc[:], in1=tmp[:])

            anchors = (2 * t, 2 * t + 1) if t < 4 else (8,)
            for a in anchors:
                half = (a % 2) * 64
                for chh in range(2):
                    g = a * 2 + chh
                    bt_ps = ppool.tile([128, K], f32, tag="btps")
                    # transpose as a plain matmul against a bf16 identity:
                    # bf16 moving operand, f32 PSUM out for the f32 b-path.
                    nc.tensor.matmul(
                        out=bt_ps[:],
                        lhsT=acc[half:half + 64, chh * 128:(chh + 1) * 128],
                        rhs=id16[half:half + 64, :],
                        start=True, stop=True,
                    )
                    app(a16h_all[:, 64 * g:64 * g + 64],
                        at16_all[:, 64 * g:64 * g + 64],
                        bt_ps[:],
                        out1_sb[:, g], out116_sb[:, g], out1h_sb[:, g],
                        halves=2 if first_app else 1)
                    first_app = False
                    # stagger: emit app2 one group late so independent app1
                    # work separates the dependent app1(g)->app2(g) stages
                    if pend_g is not None:
                        pg = pend_g
                        o2 = small.tile([128, K], f32, tag="o2")
                        app(out1h_sb[:, pg], out116_sb[:, pg],
                            rb_sb[:, CT_OFF + 64 * pg:CT_OFF + 64 * pg + 64],
                            o2[:], None, None)
                        nc.sync.dma_start(out=out_v[:, pg], in_=o2[:])
                    pend_g = g

        o2 = small.tile([128, K], f32, tag="o2", name="o2_last")
        app(out1h_sb[:, pend_g], out116_sb[:, pend_g],
            rb_sb[:, CT_OFF + 64 * pend_g:CT_OFF + 64 * pend_g + 64],
            o2[:], None, None, halves=2)
        nc.sync.dma_start(out=out_v[:, pend_g], in_=o2[:])

        for p in (cpsum, ppool, small, ecpool, relpool, gpool, singles):
            p.release()

    if not nc.is_finalized():
        nc.finalize()
    return nc


def _host_prep(inputs):
    """Per-core input maps from the full inputs (pure layout transforms)."""
    import ml_dtypes

    ra = np.asarray(inputs["rois_feature_a"], dtype=np.float32).reshape(A, N, K, C)
    rc = np.asarray(inputs["rois_feature_c"], dtype=np.float32).reshape(A, N, K, C)
    fbf = np.asarray(inputs["feature_b"], dtype=np.float32)
    wr = np.asarray(inputs["W_reg"], dtype=np.float32)
    br = np.asarray(inputs["b_reg"], dtype=np.float32)

    # conv weights: [A, C, dy, dx] -> [c_lo, (c_hi dy dx), a] flat [128, 1152]
    w = wr.transpose(1, 2, 3, 0).reshape(2, 128, BS, BS, A)
    w = w.transpose(1, 0, 2, 3, 4).reshape(128, 128 * A)

    r = (0.5 * (BS - 1) + BS * np.arange(F)).astype(np.float32)
    xc_g = np.broadcast_to(r[None, :], (F, F))
    yc_g = np.ascontiguousarray(xc_g.T)
    pad = ST * 128 - NS
    xc_s = np.concatenate([np.broadcast_to(xc_g.reshape(1, K), (A, K)).reshape(NS),
                           np.full(pad, 31.5, np.float32)]).astype(np.float32)
    yc_s = np.concatenate([np.broadcast_to(yc_g.reshape(1, K), (A, K)).reshape(NS),
                           np.full(pad, 31.5, np.float32)]).astype(np.float32)

    def to_pt(v):  # [640] -> [128, 5]
        return np.ascontiguousarray(v.reshape(ST, 128).T)

    in_maps = []
    for n in range(N):
        fbw16 = np.zeros((128, NFB16E), ml_dtypes.bfloat16)
        fbw16[:, W_OFF:W_OFF + 1152] = w.astype(ml_dtypes.bfloat16)
        fbw16[0, B_OFF:B_OFF + A] = br.astype(ml_dtypes.bfloat16)
        fbw16[0, ONE_OFF:ONE_OFF + K] = 1.0
        fb_conv = fbf[n].reshape(C, F, BS, F, BS).transpose(0, 2, 4, 1, 3)
        fbw16[:, FB_OFF:] = (fb_conv.reshape(2, 128, 8192 // 2)
                             .transpose(1, 0, 2).reshape(128, 8192)
                             .astype(ml_dtypes.bfloat16))
        fbw_h = np.frombuffer(np.ascontiguousarray(fbw16).tobytes(),
                              dtype=np.float32).reshape(128, NFBW)

        a_t = ra[:, n].transpose(0, 2, 1).reshape(GT, 128, K)   # [(a c) k]
        c_t = rc[:, n].transpose(0, 2, 1).reshape(GT, 128, K)
        at_rows = np.ascontiguousarray(a_t.transpose(1, 0, 2).reshape(128, 1152))
        ct_rows = np.ascontiguousarray(c_t.transpose(1, 0, 2).reshape(128, 1152))
        a16_pack = np.frombuffer(at_rows.astype(ml_dtypes.bfloat16).tobytes(),
                                 dtype=np.float32).reshape(128, 576)
        a16h_pack = np.frombuffer(at_rows.astype(np.float16).tobytes(),
                                  dtype=np.float32).reshape(128, 576)

        rb_h = np.zeros((128, NRB), np.float32)
        rb_h[:, CT_OFF:CT_OFF + 1152] = ct_rows
        rb_h[:, A16_OFF:A16_OFF + 576] = a16_pack
        rb_h[:, A16H_OFF:A16H_OFF + 576] = a16h_pack
        rb_h[:, XC_OFF:XC_OFF + ST] = to_pt(xc_s)
        rb_h[:, YC_OFF:YC_OFF + ST] = to_pt(yc_s)
        rb_h[:, ID_OFF:ID_OFF + 128] = np.eye(128, dtype=np.float32)
        # anchor-pair selectors: regs10[:, t] dups in cols t and t+5
        for tt in range(2 * ST):
            ae = 2 * (tt % ST)
            rb_h[ae, SE_OFF + tt] = 1.0
            if ae + 1 < A:
                rb_h[ae + 1, SO_OFF + tt] = 1.0
        # bf16 64-block identity for the acc transpose matmuls
        id16 = np.zeros((128, 64), ml_dtypes.bfloat16)
        id16[np.arange(128), np.arange(128) % 64] = 1.0
        rb_h[:, ID16_OFF:ID16_OFF + 32] = np.frombuffer(
            np.ascontiguousarray(id16).tobytes(),
            dtype=np.float32).reshape(128, 32)

        # overlapping row-pair feature table: row r = channels of pixels
        # (r, r+1), so one gather fetches both x-neighbors of a sample.
        fb_flat = np.ascontiguousarray(fbf[n].reshape(C, H * W).T)
        fbt2_n = np.ascontiguousarray(
            np.concatenate([fb_flat[:-1], fb_flat[1:]], axis=1)
        ).astype(ml_dtypes.bfloat16)
        in_maps.append({"fbw": fbw_h, "rb": rb_h, "fbt2": fbt2_n})
    return in_maps


def _assemble(results):
    """Per-core 'out' [G, K] -> full [M, C, 1, 1]."""
    outs = []
    for n in range(N):
        o = np.asarray(results[n]["out"], dtype=np.float32).reshape(A, C, K)
        outs.append(o.transpose(0, 2, 1))            # [A, K, C]
    stk = np.stack(outs, axis=1)                      # [A, N, K, C]
    return np.ascontiguousarray(stk.reshape(M, C, 1, 1))


def kernel(**inputs):
    from concourse.bass_utils import run_bass_kernel_spmd

    if "nc" not in _CACHE:
        _CACHE["nc"] = _build_nc()
    nc = _CACHE["nc"]
    in_maps = _host_prep(inputs)
    res = run_bass_kernel_spmd(nc, in_maps, core_ids=list(range(N)))
    return _assemble(res.results)



# revision 13
# speedup vs baseline: 1.0572x; 1.0137x over previous
"""AnchorToAnchor fused kernel for 8 TRN2 NeuronCores.

Shards data-parallel over the batch axis N=8 (one batch element per core).
Per core the device graph computes:
  1. block-strided conv (BoxRegress) as 129 accumulated TensorE matmuls
     (bias folded in as a rank-1 update)
  2. tanh-regressed sample centers + bilinear gather offsets/weights
  3. bilinear sampling via indirect DMA gathers from the (host-transposed)
     feature map, combined with per-partition-scalar DVE ops
  4. two anchor-to-anchor relation (softmax attention) passes with groups
     (anchor, channel) on partitions and the K x K score matrix in the free
     dimension. ScalarE expands b per-j into fp16 so the DVE outer-product
     TT runs at its 2x perf mode; exp on ScalarE (fp16 in -> bf16 out);
     e*a multiply + 3 bf16 tree-add halvings + a short tensor_reduce give
     den/num (tensor_reduce has no DVE fast mode, tree-adds do); final
     num/den combine in fp32 with a fast approximate reciprocal.

Engine notes baked into this design (measured on HW): DVE is the bottleneck
(~0.96 GHz, fp32 TT 1x, 16-bit TT 2x, single-src up to 4x; broadcast APs
with a step-0 innermost dim force 1x); GPSIMD shares SBUF ports with DVE so
offloading bulk elementwise work there is a wash; ScalarE runs ~1 elem/cyc
at 1.2 GHz for any dtype and has its own port budget, so it carries the
broadcast-expands, exps and psum copies. Compute instructions can embed only
one semaphore wait; building with bacc.Bacc legalizes multi-wait cases via
event-semaphore instructions.

The host wrapper only reshapes/transposes inputs into device-friendly
layouts (pure permutations), runs the SPMD NEFF on cores 0-7, and
re-assembles the full output.
"""

import sys

for _p in ("/opt/trn_rl_repo",):
    if _p not in sys.path:
        sys.path.insert(0, _p)

import numpy as np

# Problem constants (hardcoded per the task spec).
N, C, H, W = 8, 256, 64, 64
A, BS = 9, 8
F = H // BS          # 8
K = F * F            # 64
M = A * N * K        # 4608
ALPHA = 0.1
G = A * C            # 2304 groups per core
GT = G // 128        # 18 group tiles
ST = 5               # sample tiles of 128 (576 samples -> 4.5, padded)
NS = A * K           # 576 samples per core

# fbw16 blob: bf16 element offsets (stored as f32 words, bitcast on device)
W_OFF = 0            # conv weights [128, 128*9] bf16
B_OFF = 1152         # bias row (row 0 only) [9] bf16
ONE_OFF = 1161       # ones row (row 0 only) [64] bf16
FB_OFF = 1226        # conv feature [128, 8192] bf16 (even offset)
NFB16E = FB_OFF + 8192   # 9418 bf16 elements
NFBW = NFB16E // 2       # 4709 f32 words
WSPLIT = FB_OFF // 2     # 613 f32 words: weights+bias+ones chunk

# rb blob column offsets (f32 words)
CT_OFF = 0           # c-tensor [128, 18*64] f32
A16_OFF = 1152       # bf16 a-tensor packed [128, 576]
A16H_OFF = 1728      # fp16 a-tensor packed [128, 576]
XC_OFF = 2304        # x centers [128, 5]
YC_OFF = 2309        # y centers [128, 5]
ID_OFF = 2314        # identity [128, 128] f32
SE_OFF = 2442        # even-anchor selector [9, 10] f32
SO_OFF = 2452        # odd-anchor selector [9, 10] f32
ID16_OFF = 2462      # bf16 64-block identity [128, 64] bf16 (32 words)
NRB = 2494

_CACHE = {}


def _build_nc():
    import concourse.bass as bass
    import concourse.bacc as bacc
    import concourse.tile as tile
    from concourse import mybir

    f32 = mybir.dt.float32
    bf16 = mybir.dt.bfloat16
    f16 = mybir.dt.float16
    i32 = mybir.dt.int32
    Alu = mybir.AluOpType
    Act = mybir.ActivationFunctionType

    nc = bacc.Bacc(None)

    fbw = nc.declare_dram_parameter("fbw", [128, NFBW], f32, isOutput=False)
    rb = nc.declare_dram_parameter("rb", [128, NRB], f32, isOutput=False)
    fbt2 = nc.declare_dram_parameter("fbt2", [H * W - 1, 2 * C], bf16,
                                     isOutput=False)
    out_d = nc.declare_dram_parameter("out", [G, K], f32, isOutput=True)

    with tile.TileContext(nc) as tc:
        singles = tc.alloc_tile_pool(name="singles", bufs=1)
        gpool = tc.alloc_tile_pool(name="gpool", bufs=3)
        relpool = tc.alloc_tile_pool(name="relpool", bufs=4)
        ecpool = tc.alloc_tile_pool(name="ecpool", bufs=2)
        small = tc.alloc_tile_pool(name="small", bufs=4)
        ppool = tc.alloc_tile_pool(name="ppool", bufs=2, space="PSUM")
        cpsum = tc.alloc_tile_pool(name="cpsum", bufs=1, space="PSUM")

        # ---- resident loads; kick order mirrors consumption order ---------
        # (w first, then fb chunk halves pacing the conv, with the tiny rb
        # tail (selectors/centers) slotted early so the regs matmuls and the
        # center chain never wait on the big ct/a16 regions.)
        w_sb = singles.tile([128, WSPLIT], f32)
        nc.sync.dma_start(out=w_sb[:], in_=fbw[:, 0:WSPLIT])
        rb_sb = singles.tile([128, NRB], f32)
        fb_sb = [singles.tile([128, 1024], f32, name=f"fbc{c}")
                 for c in range(4)]
        nc.sync.dma_start(out=fb_sb[0][:, 0:512],
                          in_=fbw[:, WSPLIT:WSPLIT + 512])
        nc.sync.dma_start(out=fb_sb[0][:, 512:1024],
                          in_=fbw[:, WSPLIT + 512:WSPLIT + 1024])
        nc.sync.dma_start(out=rb_sb[:, XC_OFF:NRB], in_=rb[:, XC_OFF:NRB])
        for c in range(1, 4):
            for hh in range(2):
                s = WSPLIT + 1024 * c + 512 * hh
                nc.sync.dma_start(out=fb_sb[c][:, 512 * hh:512 * hh + 512],
                                  in_=fbw[:, s:s + 512])
        nc.sync.dma_start(out=rb_sb[:, :1152], in_=rb[:, :1152])
        nc.sync.dma_start(out=rb_sb[:, 1152:XC_OFF], in_=rb[:, 1152:XC_OFF])

        # DVE pre-touch of the rb blob: its single DMA wait lands here so
        # later DVE consumers of rb carry no fresh semaphore.
        dve_touch = singles.tile([128, 1], f32)
        nc.vector.tensor_copy(out=dve_touch[:], in_=rb_sb[:, 0:1])

        w16 = w_sb[:].bitcast(bf16)                               # [128, 1226]
        fb16 = [t[:].bitcast(bf16) for t in fb_sb]                # [128, 2048]
        at16_all = rb_sb[:, A16_OFF:A16_OFF + 576].bitcast(bf16)  # [128, 1152]
        a16h_all = rb_sb[:, A16H_OFF:A16H_OFF + 576].bitcast(f16)  # [128, 1152]
        id16 = rb_sb[:, ID16_OFF:ID16_OFF + 32].bitcast(bf16)     # [128, 64]
        xcyc = rb_sb[:, XC_OFF:XC_OFF + 2 * ST]                   # [128, 10]

        # ---- HAM warm-up: ~3.4us of dep-free dummy matmuls so the conv and
        # the first transposes run at 2.4 GHz instead of the cold 1.2 --------
        dummy = singles.tile([128, 256], f32)
        nc.vector.memset(dummy[:], 0.0)
        dummy16 = dummy[:].bitcast(bf16)
        warm_ps = cpsum.tile([128, 384], f32)
        for i in range(8):
            nc.tensor.matmul(out=warm_ps[:], lhsT=dummy16[:, 0:128],
                             rhs=dummy16[:, 0:384], start=(i == 0),
                             stop=(i == 7))

        # ---- conv (BoxRegress) in bf16, out [a, ij]; per-chunk DMA deps ----
        conv_ps = cpsum.tile([A, K], f32)
        for k in range(128):
            c, kk = divmod(k, 32)
            nc.tensor.matmul(
                out=conv_ps[:],
                lhsT=w16[:, W_OFF + 9 * k:W_OFF + 9 * k + 9],
                rhs=fb16[c][:, 64 * kk:64 * kk + 64],
                start=(k == 0),
                stop=False,
            )
        nc.tensor.matmul(
            out=conv_ps[:],
            lhsT=w16[0:1, B_OFF:B_OFF + A],
            rhs=w16[0:1, ONE_OFF:ONE_OFF + K],
            start=False,
            stop=True,
        )
        conv_s = singles.tile([A, K], f32)
        nc.scalar.copy(out=conv_s[:], in_=conv_ps[:])

        # reorg [a, ij] -> regs10[(a ij) % 128, dup((a ij) // 128)] via PE
        # selector matmuls (no tiny DMAs); columns 0-4 == 5-9 so the x/y
        # center chains below run joint on [128, 10] tiles.
        regs_ps = cpsum.tile([128, 2 * ST], f32)
        nc.tensor.matmul(out=regs_ps[0:64, :], lhsT=conv_s[:],
                         rhs=rb_sb[0:A, SE_OFF:SE_OFF + 2 * ST],
                         start=True, stop=True)
        nc.tensor.matmul(out=regs_ps[64:128, :], lhsT=conv_s[:],
                         rhs=rb_sb[0:A, SO_OFF:SO_OFF + 2 * ST],
                         start=True, stop=True, tile_position=(0, 64))

        # ---- centers, offsets, weights (joint x/y on [128, 10]) ------------
        th10 = small.tile([128, 2 * ST], f32)
        nc.scalar.activation(out=th10[:], in_=regs_ps[:], func=Act.Tanh)
        pxy = small.tile([128, 2 * ST], f32)
        nc.vector.scalar_tensor_tensor(out=pxy[:], in0=th10[:],
                                       scalar=ALPHA * BS, in1=xcyc,
                                       op0=Alu.mult, op1=Alu.add)

        ri = small.tile([128, 2 * ST], i32)
        nc.vector.tensor_copy(out=ri[:], in_=pxy[:])
        rf = small.tile([128, 2 * ST], f32)
        nc.vector.tensor_copy(out=rf[:], in_=ri[:])
        gt = small.tile([128, 2 * ST], f32)
        nc.vector.tensor_tensor(out=gt[:], in0=rf[:], in1=pxy[:], op=Alu.is_gt)
        xy0f = small.tile([128, 2 * ST], f32)
        nc.vector.tensor_sub(out=xy0f[:], in0=rf[:], in1=gt[:])
        # uwxy = [ux, uy, wx, wy]: adjacency lets the q-products batch in 2 TTs
        uwxy = small.tile([128, 4 * ST], f32)
        nc.vector.tensor_sub(out=uwxy[:, 2 * ST:4 * ST], in0=pxy[:],
                             in1=xy0f[:])
        nc.vector.tensor_scalar(out=uwxy[:, 0:2 * ST],
                                in0=uwxy[:, 2 * ST:4 * ST], scalar1=-1.0,
                                scalar2=1.0, op0=Alu.mult, op1=Alu.add)

        o00f = small.tile([128, ST], f32)
        nc.vector.scalar_tensor_tensor(out=o00f[:], in0=xy0f[:, ST:2 * ST],
                                       scalar=float(W), in1=xy0f[:, 0:ST],
                                       op0=Alu.mult, op1=Alu.add)
        oi0 = small.tile([128, ST], i32)
        nc.vector.tensor_copy(out=oi0[:], in_=o00f[:])
        o64f = small.tile([128, ST], f32)
        nc.vector.tensor_scalar_add(o64f[:], o00f[:], float(W))
        oi64 = small.tile([128, ST], i32)
        nc.vector.tensor_copy(out=oi64[:], in_=o64f[:])

        # bilinear weight products for all tiles at once: [128, 20] q-major
        # ([ux|wx] x uy-bcast, then x wy-bcast)
        uw_v = uwxy[:].rearrange("p (a b t) -> p a b t", a=2, b=2)
        wq20 = small.tile([128, 4 * ST], f32)
        wq_v = wq20[:].rearrange("p (a t) -> p a t", a=4)
        nc.vector.tensor_tensor(
            out=wq_v[:, 0:2],
            in0=uw_v[:, :, 0],
            in1=uw_v[:, 0:1, 1].to_broadcast([128, 2, ST]),
            op=Alu.mult)
        nc.vector.tensor_tensor(
            out=wq_v[:, 2:4],
            in0=uw_v[:, :, 0],
            in1=uw_v[:, 1:2, 1].to_broadcast([128, 2, ST]),
            op=Alu.mult)

        # ---- per sample-tile gather + bilinear; per anchor transpose + apps -
        out1_sb = singles.tile([128, GT, K], f32)
        out116_sb = singles.tile([128, GT, K], bf16)
        out1h_sb = singles.tile([128, GT, K], f16)
        pend_g = None
        out_v = out_d.rearrange("(g p) k -> p g k", p=128)

        def app(a_h, a_b, b_f, o_f, o_b, o_h, halves=1):
            # ScalarE materializes the per-j broadcast of b in fp16 so the
            # DVE outer-product TT has step-1 fp16 operands and runs at 2x.
            # halves=2 splits along j to pipeline ScalarE/DVE stages when
            # this app is alone in flight (pipeline head / tail).
            bexp = relpool.tile([128, K, K], f16, tag="bexp")
            rel = relpool.tile([128, K, K], f16, tag="rel")
            ec = ecpool.tile([128, 2, K, K], bf16, tag="ec")
            t0 = ecpool.tile([128, 2, K, 32], bf16, tag="t0")
            t1 = ecpool.tile([128, 2, K, 16], bf16, tag="t1")
            t2 = ecpool.tile([128, 2, K, 8], bf16, tag="t2")
            t3 = ecpool.tile([128, 2, K, 4], bf16, tag="t3")
            t4 = ecpool.tile([128, 2, K, 2], bf16, tag="t4")
            dn = small.tile([128, 2, K], f32, tag="dn")
            inv = small.tile([128, K], f32, tag="inv")
            r = small.tile([128, K], f32, tag="r")
            jw = K // halves
            for h in range(halves):
                j0, j1 = h * jw, (h + 1) * jw
                nc.scalar.activation(
                    out=bexp[:, j0:j1],
                    in_=b_f[:, j0:j1].unsqueeze(2).to_broadcast([128, jw, K]),
                    func=Act.Copy)
                nc.vector.tensor_tensor(
                    out=rel[:, j0:j1],
                    in0=a_h.unsqueeze(1).to_broadcast([128, jw, K]),
                    in1=bexp[:, j0:j1],
                    op=Alu.mult,
                )
                nc.scalar.activation(out=ec[:, 0, j0:j1], in_=rel[:, j0:j1],
                                     func=Act.Exp)
                nc.vector.tensor_tensor(
                    out=ec[:, 1, j0:j1],
                    in0=ec[:, 0, j0:j1],
                    in1=a_b.unsqueeze(1).to_broadcast([128, jw, K]),
                    op=Alu.mult,
                )
                # bf16 tree-adds run at 2x on DVE while tensor_reduce is stuck
                # at 1x; halving all the way to width 1 wins.
                nc.vector.tensor_tensor(out=t0[:, :, j0:j1],
                                        in0=ec[:, :, j0:j1, 0:32],
                                        in1=ec[:, :, j0:j1, 32:64], op=Alu.add)
                nc.vector.tensor_tensor(out=t1[:, :, j0:j1],
                                        in0=t0[:, :, j0:j1, 0:16],
                                        in1=t0[:, :, j0:j1, 16:32], op=Alu.add)
                nc.vector.tensor_tensor(out=t2[:, :, j0:j1],
                                        in0=t1[:, :, j0:j1, 0:8],
                                        in1=t1[:, :, j0:j1, 8:16], op=Alu.add)
                nc.vector.tensor_tensor(out=t3[:, :, j0:j1],
                                        in0=t2[:, :, j0:j1, 0:4],
                                        in1=t2[:, :, j0:j1, 4:8], op=Alu.add)
                nc.vector.tensor_tensor(out=t4[:, :, j0:j1],
                                        in0=t3[:, :, j0:j1, 0:2],
                                        in1=t3[:, :, j0:j1, 2:4], op=Alu.add)
                nc.vector.tensor_tensor(out=dn[:, :, j0:j1],
                                        in0=t4[:, :, j0:j1, 0],
                                        in1=t4[:, :, j0:j1, 1], op=Alu.add)
                nc.vector.reciprocal_approx_fast(out=inv[:, j0:j1],
                                                 in_=dn[:, 0, j0:j1])
                nc.vector.tensor_mul(out=r[:, j0:j1], in0=dn[:, 1, j0:j1],
                                     in1=inv[:, j0:j1])
                nc.vector.tensor_add(out=o_f[:, j0:j1], in0=r[:, j0:j1],
                                     in1=b_f[:, j0:j1])
                if o_b is not None:
                    nc.scalar.copy(out=o_b[:, j0:j1], in_=o_f[:, j0:j1])
                    # fp16 copy on DVE: app2's rel follows in-engine order, no
                    # ScalarE round-trip on the serial app1->app2 chain
                    nc.vector.tensor_copy(out=o_h[:, j0:j1], in_=o_f[:, j0:j1])

        first_app = True
        for t in range(ST):
            # two row-pair gathers per tile: fbt2 row r = [pixel r | pixel
            # r+1] channels, so (y0,x0)+(y0,x1) come in one descriptor set.
            v0 = gpool.tile([128, 2 * C], bf16, tag="v0")
            nc.gpsimd.indirect_dma_start(
                out=v0[:], out_offset=None, in_=fbt2[:],
                in_offset=bass.IndirectOffsetOnAxis(ap=oi0[:, t:t + 1], axis=0),
            )
            v1 = gpool.tile([128, 2 * C], bf16, tag="v1")
            nc.gpsimd.indirect_dma_start(
                out=v1[:], out_offset=None, in_=fbt2[:],
                in_offset=bass.IndirectOffsetOnAxis(ap=oi64[:, t:t + 1], axis=0),
            )
            # per-partition bilinear weights ride ScalarE's activation scale;
            # during the pipeline ramp (tiles 0-1) DVE is idle and ScalarE
            # gates the first apps, so run them as DVE tensor_scalar instead.
            sc = gpool.tile([128, 4, C], bf16, tag="sc")
            srcs = (v0[:, 0:C], v0[:, C:2 * C], v1[:, 0:C], v1[:, C:2 * C])
            for q in range(4):
                wcol = wq20[:, ST * q + t:ST * q + t + 1]
                if t < 2:
                    nc.vector.tensor_scalar_mul(sc[:, q], srcs[q], wcol)
                else:
                    nc.scalar.activation(out=sc[:, q], in_=srcs[q],
                                         func=Act.Copy, scale=wcol)
            acc = gpool.tile([128, C], bf16, tag="acc")
            tmp = gpool.tile([128, C], bf16, tag="tmp")
            nc.vector.tensor_add(out=tmp[:], in0=sc[:, 0], in1=sc[:, 1])
            nc.vector.tensor_add(out=acc[:], in0=sc[:, 2], in1=sc[:, 3])
            nc.vector.tensor_add(out=acc[:], in0=acc[:], in1=tmp[:])

            anchors = (2 * t, 2 * t + 1) if t < 4 else (8,)
            for a in anchors:
                half = (a % 2) * 64
                for chh in range(2):
                    g = a * 2 + chh
                    bt_ps = ppool.tile([128, K], f32, tag="btps")
                    # transpose as a plain matmul against a bf16 identity:
                    # bf16 moving operand, f32 PSUM out for the f32 b-path.
                    nc.tensor.matmul(
                        out=bt_ps[:],
                        lhsT=acc[half:half + 64, chh * 128:(chh + 1) * 128],
                        rhs=id16[half:half + 64, :],
                        start=True, stop=True,
                    )
                    app(a16h_all[:, 64 * g:64 * g + 64],
                        at16_all[:, 64 * g:64 * g + 64],
                        bt_ps[:],
                        out1_sb[:, g], out116_sb[:, g], out1h_sb[:, g],
                        halves=2 if first_app else 1)
                    first_app = False
                    # stagger: emit app2 one group late so independent app1
                    # work separates the dependent app1(g)->app2(g) stages
                    if pend_g is not None:
                        pg = pend_g
                        o2 = small.tile([128, K], f32, tag="o2")
                        app(out1h_sb[:, pg], out116_sb[:, pg],
                            rb_sb[:, CT_OFF + 64 * pg:CT_OFF + 64 * pg + 64],
                            o2[:], None, None)
                        nc.sync.dma_start(out=out_v[:, pg], in_=o2[:])
                    pend_g = g

        o2 = small.tile([128, K], f32, tag="o2", name="o2_last")
        app(out1h_sb[:, pend_g], out116_sb[:, pend_g],
            rb_sb[:, CT_OFF + 64 * pend_g:CT_OFF + 64 * pend_g + 64],
            o2[:], None, None, halves=2)
        nc.sync.dma_start(out=out_v[:, pend_g], in_=o2[:])

        for p in (cpsum, ppool, small, ecpool, relpool, gpool, singles):
            p.release()

    if not nc.is_finalized():
        nc.finalize()
    return nc


def _host_prep(inputs):
    """Per-core input maps from the full inputs (pure layout transforms)."""
    import ml_dtypes

    ra = np.asarray(inputs["rois_feature_a"], dtype=np.float32).reshape(A, N, K, C)
    rc = np.asarray(inputs["rois_feature_c"], dtype=np.float32).reshape(A, N, K, C)
    fbf = np.asarray(inputs["feature_b"], dtype=np.float32)
    wr = np.asarray(inputs["W_reg"], dtype=np.float32)
    br = np.asarray(inputs["b_reg"], dtype=np.float32)

    # conv weights: [A, C, dy, dx] -> [c_lo, (c_hi dy dx), a] flat [128, 1152]
    w = wr.transpose(1, 2, 3, 0).reshape(2, 128, BS, BS, A)
    w = w.transpose(1, 0, 2, 3, 4).reshape(128, 128 * A)

    r = (0.5 * (BS - 1) + BS * np.arange(F)).astype(np.float32)
    xc_g = np.broadcast_to(r[None, :], (F, F))
    yc_g = np.ascontiguousarray(xc_g.T)
    pad = ST * 128 - NS
    xc_s = np.concatenate([np.broadcast_to(xc_g.reshape(1, K), (A, K)).reshape(NS),
                           np.full(pad, 31.5, np.float32)]).astype(np.float32)
    yc_s = np.concatenate([np.broadcast_to(yc_g.reshape(1, K), (A, K)).reshape(NS),
                           np.full(pad, 31.5, np.float32)]).astype(np.float32)

    def to_pt(v):  # [640] -> [128, 5]
        return np.ascontiguousarray(v.reshape(ST, 128).T)

    in_maps = []
    for n in range(N):
        fbw16 = np.zeros((128, NFB16E), ml_dtypes.bfloat16)
        fbw16[:, W_OFF:W_OFF + 1152] = w.astype(ml_dtypes.bfloat16)
        fbw16[0, B_OFF:B_OFF + A] = br.astype(ml_dtypes.bfloat16)
        fbw16[0, ONE_OFF:ONE_OFF + K] = 1.0
        fb_conv = fbf[n].reshape(C, F, BS, F, BS).transpose(0, 2, 4, 1, 3)
        fbw16[:, FB_OFF:] = (fb_conv.reshape(2, 128, 8192 // 2)
                             .transpose(1, 0, 2).reshape(128, 8192)
                             .astype(ml_dtypes.bfloat16))
        fbw_h = np.frombuffer(np.ascontiguousarray(fbw16).tobytes(),
                              dtype=np.float32).reshape(128, NFBW)

        a_t = ra[:, n].transpose(0, 2, 1).reshape(GT, 128, K)   # [(a c) k]
        c_t = rc[:, n].transpose(0, 2, 1).reshape(GT, 128, K)
        at_rows = np.ascontiguousarray(a_t.transpose(1, 0, 2).reshape(128, 1152))
        ct_rows = np.ascontiguousarray(c_t.transpose(1, 0, 2).reshape(128, 1152))
        a16_pack = np.frombuffer(at_rows.astype(ml_dtypes.bfloat16).tobytes(),
                                 dtype=np.float32).reshape(128, 576)
        a16h_pack = np.frombuffer(at_rows.astype(np.float16).tobytes(),
                                  dtype=np.float32).reshape(128, 576)

        rb_h = np.zeros((128, NRB), np.float32)
        rb_h[:, CT_OFF:CT_OFF + 1152] = ct_rows
        rb_h[:, A16_OFF:A16_OFF + 576] = a16_pack
        rb_h[:, A16H_OFF:A16H_OFF + 576] = a16h_pack
        rb_h[:, XC_OFF:XC_OFF + ST] = to_pt(xc_s)
        rb_h[:, YC_OFF:YC_OFF + ST] = to_pt(yc_s)
        rb_h[:, ID_OFF:ID_OFF + 128] = np.eye(128, dtype=np.float32)
        # anchor-pair selectors: regs10[:, t] dups in cols t and t+5
        for tt in range(2 * ST):
            ae = 2 * (tt % ST)
            rb_h[ae, SE_OFF + tt] = 1.0
            if ae + 1 < A:
                rb_h[ae + 1, SO_OFF + tt] = 1.0
        # bf16 64-block identity for the acc transpose matmuls
        id16 = np.zeros((128, 64), ml_dtypes.bfloat16)
        id16[np.arange(128), np.arange(128) % 64] = 1.0
        rb_h[:, ID16_OFF:ID16_OFF + 32] = np.frombuffer(
            np.ascontiguousarray(id16).tobytes(),
            dtype=np.float32).reshape(128, 32)

        # overlapping row-pair feature table: row r = channels of pixels
        # (r, r+1), so one gather fetches both x-neighbors of a sample.
        fb_flat = np.ascontiguousarray(fbf[n].reshape(C, H * W).T)
        fbt2_n = np.ascontiguousarray(
            np.concatenate([fb_flat[:-1], fb_flat[1:]], axis=1)
        ).astype(ml_dtypes.bfloat16)
        in_maps.append({"fbw": fbw_h, "rb": rb_h, "fbt2": fbt2_n})
    return in_maps


def _assemble(results):
    """Per-core 'out' [G, K] -> full [M, C, 1, 1]."""
    outs = []
    for n in range(N):
        o = np.asarray(results[n]["out"], dtype=np.float32).reshape(A, C, K)
        outs.append(o.transpose(0, 2, 1))            # [A, K, C]
    stk = np.stack(outs, axis=1)                      # [A, N, K, C]
    return np.ascontiguousarray(stk.reshape(M, C, 1, 1))


def kernel(**inputs):
    from concourse.bass_utils import run_bass_kernel_spmd

    if "nc" not in _CACHE:
        _CACHE["nc"] = _build_nc()
    nc = _CACHE["nc"]
    in_maps = _host_prep(inputs)
    res = run_bass_kernel_spmd(nc, in_maps, core_ids=list(range(N)))
    return _assemble(res.results)



# revision 17
# speedup vs baseline: 1.0767x; 1.0184x over previous
"""AnchorToAnchor fused kernel for 8 TRN2 NeuronCores.

Shards data-parallel over the batch axis N=8 (one batch element per core).
Per core the device graph computes:
  1. block-strided conv (BoxRegress) as 129 accumulated TensorE matmuls
     (bias folded in as a rank-1 update)
  2. tanh-regressed sample centers + bilinear gather offsets/weights
  3. bilinear sampling via indirect DMA gathers from the (host-transposed)
     feature map, combined with per-partition-scalar DVE ops
  4. two anchor-to-anchor relation (softmax attention) passes with groups
     (anchor, channel) on partitions and the K x K score matrix in the free
     dimension. ScalarE expands b per-j into fp16 so the DVE outer-product
     TT runs at its 2x perf mode; exp on ScalarE (fp16 in -> bf16 out);
     e*a multiply + 3 bf16 tree-add halvings + a short tensor_reduce give
     den/num (tensor_reduce has no DVE fast mode, tree-adds do); final
     num/den combine in fp32 with a fast approximate reciprocal.

Engine notes baked into this design (measured on HW): DVE is the bottleneck
(~0.96 GHz, fp32 TT 1x, 16-bit TT 2x, single-src up to 4x; broadcast APs
with a step-0 innermost dim force 1x); GPSIMD shares SBUF ports with DVE so
offloading bulk elementwise work there is a wash; ScalarE runs ~1 elem/cyc
at 1.2 GHz for any dtype and has its own port budget, so it carries the
broadcast-expands, exps and psum copies. Compute instructions can embed only
one semaphore wait; building with bacc.Bacc legalizes multi-wait cases via
event-semaphore instructions.

The host wrapper only reshapes/transposes inputs into device-friendly
layouts (pure permutations), runs the SPMD NEFF on cores 0-7, and
re-assembles the full output.
"""

import sys

for _p in ("/opt/trn_rl_repo",):
    if _p not in sys.path:
        sys.path.insert(0, _p)

import numpy as np

# Problem constants (hardcoded per the task spec).
N, C, H, W = 8, 256, 64, 64
A, BS = 9, 8
F = H // BS          # 8
K = F * F            # 64
M = A * N * K        # 4608
ALPHA = 0.1
G = A * C            # 2304 groups per core
GT = G // 128        # 18 group tiles
ST = 5               # sample tiles of 128 (576 samples -> 4.5, padded)
NS = A * K           # 576 samples per core

# fbw16 blob: bf16 element offsets (stored as f32 words, bitcast on device)
W_OFF = 0            # conv weights [128, 128*9] bf16
B_OFF = 1152         # bias row (row 0 only) [9] bf16
ONE_OFF = 1161       # ones row (row 0 only) [64] bf16
FB_OFF = 1226        # conv feature [128, 8192] bf16 (even offset)
NFB16E = FB_OFF + 8192   # 9418 bf16 elements
NFBW = NFB16E // 2       # 4709 f32 words
WSPLIT = FB_OFF // 2     # 613 f32 words: weights+bias+ones chunk

# rb blob column offsets (f32 words)
CT_OFF = 0           # c-tensor [128, 18*64] f32
A16_OFF = 1152       # bf16 a-tensor packed [128, 576]
A16H_OFF = 1728      # fp16 a-tensor packed [128, 576]
XC_OFF = 2304        # x centers [128, 5]
YC_OFF = 2309        # y centers [128, 5]
ID_OFF = 2314        # identity [128, 128] f32
SE_OFF = 2442        # even-anchor selector [9, 10] f32
SO_OFF = 2452        # odd-anchor selector [9, 10] f32
ID16_OFF = 2462      # bf16 64-block identity [128, 64] bf16 (32 words)
NRB = 2494

_CACHE = {}


def _build_nc():
    import concourse.bass as bass
    import concourse.bacc as bacc
    import concourse.tile as tile
    from concourse import mybir

    f32 = mybir.dt.float32
    bf16 = mybir.dt.bfloat16
    f16 = mybir.dt.float16
    i32 = mybir.dt.int32
    Alu = mybir.AluOpType
    Act = mybir.ActivationFunctionType

    nc = bacc.Bacc(None)

    fbw = nc.declare_dram_parameter("fbw", [128, NFBW], f32, isOutput=False)
    rb = nc.declare_dram_parameter("rb", [128, NRB], f32, isOutput=False)
    fbt2 = nc.declare_dram_parameter("fbt2", [H * W - 1, 2 * C], bf16,
                                     isOutput=False)
    out_d = nc.declare_dram_parameter("out", [G, K], f32, isOutput=True)

    with tile.TileContext(nc) as tc:
        singles = tc.alloc_tile_pool(name="singles", bufs=1)
        gpool = tc.alloc_tile_pool(name="gpool", bufs=3)
        relpool = tc.alloc_tile_pool(name="relpool", bufs=4)
        ecpool = tc.alloc_tile_pool(name="ecpool", bufs=2)
        small = tc.alloc_tile_pool(name="small", bufs=4)
        ppool = tc.alloc_tile_pool(name="ppool", bufs=2, space="PSUM")
        cpsum = tc.alloc_tile_pool(name="cpsum", bufs=1, space="PSUM")

        # ---- resident loads; kick order mirrors consumption order ---------
        # (w first, then fb chunk halves pacing the conv, with the tiny rb
        # tail (selectors/centers) slotted early so the regs matmuls and the
        # center chain never wait on the big ct/a16 regions.)
        # HWDGE has two queues (SP + Activation engines); alternate the chunk
        # halves across both so transfers run 2-way parallel while ScalarE is
        # otherwise idle at the head.
        w_sb = singles.tile([128, WSPLIT], f32)
        nc.sync.dma_start(out=w_sb[:], in_=fbw[:, 0:WSPLIT])
        rb_sb = singles.tile([128, NRB], f32)
        nc.scalar.dma_start(out=rb_sb[:, XC_OFF:NRB], in_=rb[:, XC_OFF:NRB])
        fb_sb = [singles.tile([128, 1024], f32, name=f"fbc{c}")
                 for c in range(4)]
        for c in range(4):
            s = WSPLIT + 1024 * c
            nc.sync.dma_start(out=fb_sb[c][:, 0:512], in_=fbw[:, s:s + 512])
            nc.scalar.dma_start(out=fb_sb[c][:, 512:1024],
                                in_=fbw[:, s + 512:s + 1024])
        nc.sync.dma_start(out=rb_sb[:, :1152], in_=rb[:, :1152])
        nc.scalar.dma_start(out=rb_sb[:, 1152:XC_OFF], in_=rb[:, 1152:XC_OFF])

        # DVE pre-touch of the rb blob: its single DMA wait lands here so
        # later DVE consumers of rb carry no fresh semaphore.
        dve_touch = singles.tile([128, 1], f32)
        nc.vector.tensor_copy(out=dve_touch[:], in_=rb_sb[:, 0:1])

        w16 = w_sb[:].bitcast(bf16)                               # [128, 1226]
        fb16 = [t[:].bitcast(bf16) for t in fb_sb]                # [128, 2048]
        at16_all = rb_sb[:, A16_OFF:A16_OFF + 576].bitcast(bf16)  # [128, 1152]
        a16h_all = rb_sb[:, A16H_OFF:A16H_OFF + 576].bitcast(f16)  # [128, 1152]
        id16 = rb_sb[:, ID16_OFF:ID16_OFF + 32].bitcast(bf16)     # [128, 64]
        xcyc = rb_sb[:, XC_OFF:XC_OFF + 2 * ST]                   # [128, 10]

        # ---- HAM warm-up: ~3.4us of dep-free dummy matmuls so the conv and
        # the first transposes run at 2.4 GHz instead of the cold 1.2 --------
        dummy = singles.tile([128, 256], f32)
        nc.vector.memset(dummy[:], 0.0)
        dummy16 = dummy[:].bitcast(bf16)
        warm_ps = cpsum.tile([128, 384], f32)
        for i in range(6):
            nc.tensor.matmul(out=warm_ps[:], lhsT=dummy16[:, 0:128],
                             rhs=dummy16[:, 0:384], start=(i == 0),
                             stop=(i == 5))

        # ---- conv (BoxRegress) in bf16, out [a, ij]; per-chunk DMA deps ----
        conv_ps = cpsum.tile([A, K], f32)
        for k in range(128):
            c, kk = divmod(k, 32)
            nc.tensor.matmul(
                out=conv_ps[:],
                lhsT=w16[:, W_OFF + 9 * k:W_OFF + 9 * k + 9],
                rhs=fb16[c][:, 64 * kk:64 * kk + 64],
                start=(k == 0),
                stop=False,
            )
        nc.tensor.matmul(
            out=conv_ps[:],
            lhsT=w16[0:1, B_OFF:B_OFF + A],
            rhs=w16[0:1, ONE_OFF:ONE_OFF + K],
            start=False,
            stop=True,
        )
        conv_s = singles.tile([A, K], f32)
        nc.scalar.copy(out=conv_s[:], in_=conv_ps[:])

        # reorg [a, ij] -> regs10[(a ij) % 128, dup((a ij) // 128)] via PE
        # selector matmuls (no tiny DMAs); columns 0-4 == 5-9 so the x/y
        # center chains below run joint on [128, 10] tiles.
        regs_ps = cpsum.tile([128, 2 * ST], f32)
        nc.tensor.matmul(out=regs_ps[0:64, :], lhsT=conv_s[:],
                         rhs=rb_sb[0:A, SE_OFF:SE_OFF + 2 * ST],
                         start=True, stop=True)
        nc.tensor.matmul(out=regs_ps[64:128, :], lhsT=conv_s[:],
                         rhs=rb_sb[0:A, SO_OFF:SO_OFF + 2 * ST],
                         start=True, stop=True, tile_position=(0, 64))

        # ---- centers, offsets, weights (joint x/y on [128, 10]) ------------
        th10 = small.tile([128, 2 * ST], f32)
        nc.scalar.activation(out=th10[:], in_=regs_ps[:], func=Act.Tanh)
        pxy = small.tile([128, 2 * ST], f32)
        nc.vector.scalar_tensor_tensor(out=pxy[:], in0=th10[:],
                                       scalar=ALPHA * BS, in1=xcyc,
                                       op0=Alu.mult, op1=Alu.add)

        ri = small.tile([128, 2 * ST], i32)
        nc.vector.tensor_copy(out=ri[:], in_=pxy[:])
        rf = small.tile([128, 2 * ST], f32)
        nc.vector.tensor_copy(out=rf[:], in_=ri[:])
        gt = small.tile([128, 2 * ST], f32)
        nc.vector.tensor_tensor(out=gt[:], in0=rf[:], in1=pxy[:], op=Alu.is_gt)
        xy0f = small.tile([128, 2 * ST], f32)
        nc.vector.tensor_sub(out=xy0f[:], in0=rf[:], in1=gt[:])
        # uwxy = [ux, uy, wx, wy]: adjacency lets the q-products batch in 2 TTs
        uwxy = small.tile([128, 4 * ST], f32)
        nc.vector.tensor_sub(out=uwxy[:, 2 * ST:4 * ST], in0=pxy[:],
                             in1=xy0f[:])
        nc.vector.tensor_scalar(out=uwxy[:, 0:2 * ST],
                                in0=uwxy[:, 2 * ST:4 * ST], scalar1=-1.0,
                                scalar2=1.0, op0=Alu.mult, op1=Alu.add)

        o00f = small.tile([128, ST], f32)
        nc.vector.scalar_tensor_tensor(out=o00f[:], in0=xy0f[:, ST:2 * ST],
                                       scalar=float(W), in1=xy0f[:, 0:ST],
                                       op0=Alu.mult, op1=Alu.add)
        oi0 = small.tile([128, ST], i32)
        nc.vector.tensor_copy(out=oi0[:], in_=o00f[:])
        o64f = small.tile([128, ST], f32)
        nc.vector.tensor_scalar_add(o64f[:], o00f[:], float(W))
        oi64 = small.tile([128, ST], i32)
        nc.vector.tensor_copy(out=oi64[:], in_=o64f[:])

        # bilinear weight products for all tiles at once: [128, 20] q-major
        # ([ux|wx] x uy-bcast, then x wy-bcast)
        uw_v = uwxy[:].rearrange("p (a b t) -> p a b t", a=2, b=2)
        wq20 = small.tile([128, 4 * ST], f32)
        wq_v = wq20[:].rearrange("p (a t) -> p a t", a=4)
        nc.vector.tensor_tensor(
            out=wq_v[:, 0:2],
            in0=uw_v[:, :, 0],
            in1=uw_v[:, 0:1, 1].to_broadcast([128, 2, ST]),
            op=Alu.mult)
        nc.vector.tensor_tensor(
            out=wq_v[:, 2:4],
            in0=uw_v[:, :, 0],
            in1=uw_v[:, 1:2, 1].to_broadcast([128, 2, ST]),
            op=Alu.mult)

        # ---- per sample-tile gather + bilinear; per anchor transpose + apps -
        out1_sb = singles.tile([128, GT, K], f32)
        out116_sb = singles.tile([128, GT, K], bf16)
        out1h_sb = singles.tile([128, GT, K], f16)
        pend_g = None
        out_v = out_d.rearrange("(g p) k -> p g k", p=128)

        def app(a_h, a_b, b_f, dn_out, halves=1):
            # ScalarE materializes the per-j broadcast of b in fp16 so the
            # DVE outer-product TT has step-1 fp16 operands and runs at 2x.
            # halves=2 splits along j to pipeline ScalarE/DVE stages when
            # this app is alone in flight (pipeline head / tail).
            # The num/den sums land in dn_out [128, 2, K]; the caller batches
            # the small combine ops across a group pair (see combine2).
            bexp = relpool.tile([128, K, K], f16, tag="bexp")
            rel = relpool.tile([128, K, K], f16, tag="rel")
            ec = ecpool.tile([128, 2, K, K], bf16, tag="ec")
            t0 = ecpool.tile([128, 2, K, 32], bf16, tag="t0")
            t1 = ecpool.tile([128, 2, K, 16], bf16, tag="t1")
            t2 = ecpool.tile([128, 2, K, 8], bf16, tag="t2")
            t3 = ecpool.tile([128, 2, K, 4], bf16, tag="t3")
            t4 = ecpool.tile([128, 2, K, 2], bf16, tag="t4")
            jw = K // halves
            for h in range(halves):
                j0, j1 = h * jw, (h + 1) * jw
                nc.scalar.activation(
                    out=bexp[:, j0:j1],
                    in_=b_f[:, j0:j1].unsqueeze(2).to_broadcast([128, jw, K]),
                    func=Act.Copy)
                nc.vector.tensor_tensor(
                    out=rel[:, j0:j1],
                    in0=a_h.unsqueeze(1).to_broadcast([128, jw, K]),
                    in1=bexp[:, j0:j1],
                    op=Alu.mult,
                )
                nc.scalar.activation(out=ec[:, 0, j0:j1], in_=rel[:, j0:j1],
                                     func=Act.Exp)
                nc.vector.tensor_tensor(
                    out=ec[:, 1, j0:j1],
                    in0=ec[:, 0, j0:j1],
                    in1=a_b.unsqueeze(1).to_broadcast([128, jw, K]),
                    op=Alu.mult,
                )
                # bf16 tree-adds run at 2x on DVE while tensor_reduce is stuck
                # at 1x; halving all the way to width 1 wins.
                nc.vector.tensor_tensor(out=t0[:, :, j0:j1],
                                        in0=ec[:, :, j0:j1, 0:32],
                                        in1=ec[:, :, j0:j1, 32:64], op=Alu.add)
                nc.vector.tensor_tensor(out=t1[:, :, j0:j1],
                                        in0=t0[:, :, j0:j1, 0:16],
                                        in1=t0[:, :, j0:j1, 16:32], op=Alu.add)
                nc.vector.tensor_tensor(out=t2[:, :, j0:j1],
                                        in0=t1[:, :, j0:j1, 0:8],
                                        in1=t1[:, :, j0:j1, 8:16], op=Alu.add)
                nc.vector.tensor_tensor(out=t3[:, :, j0:j1],
                                        in0=t2[:, :, j0:j1, 0:4],
                                        in1=t2[:, :, j0:j1, 4:8], op=Alu.add)
                nc.vector.tensor_tensor(out=t4[:, :, j0:j1],
                                        in0=t3[:, :, j0:j1, 0:2],
                                        in1=t3[:, :, j0:j1, 2:4], op=Alu.add)
                nc.vector.tensor_tensor(out=dn_out[:, :, j0:j1],
                                        in0=t4[:, :, j0:j1, 0],
                                        in1=t4[:, :, j0:j1, 1], op=Alu.add)

        def combine2(dn2, b2, o_f2, o_b2, o_h2):
            # one recip/mult/add (+copies) over BOTH groups of an anchor:
            # [128, 2, K]-shaped ops halve the per-instruction overhead of
            # the combine stage.
            inv2 = small.tile([128, 2, K], f32, tag="inv2")
            nc.vector.reciprocal_approx_fast(out=inv2[:], in_=dn2[:, :, 0])
            r2 = small.tile([128, 2, K], f32, tag="r2")
            nc.vector.tensor_mul(out=r2[:], in0=dn2[:, :, 1], in1=inv2[:])
            nc.vector.tensor_add(out=o_f2[:], in0=r2[:], in1=b2)
            if o_b2 is not None:
                nc.scalar.copy(out=o_b2[:], in_=o_f2[:])
                # fp16 copy on DVE: app2's rel follows in-engine order, no
                # ScalarE round-trip on the serial app1->app2 chain
                nc.vector.tensor_copy(out=o_h2[:], in_=o_f2[:])

        first_app = True
        for t in range(ST):
            # two row-pair gathers per tile: fbt2 row r = [pixel r | pixel
            # r+1] channels, so (y0,x0)+(y0,x1) come in one descriptor set.
            v0 = gpool.tile([128, 2 * C], bf16, tag="v0")
            nc.gpsimd.indirect_dma_start(
                out=v0[:], out_offset=None, in_=fbt2[:],
                in_offset=bass.IndirectOffsetOnAxis(ap=oi0[:, t:t + 1], axis=0),
            )
            v1 = gpool.tile([128, 2 * C], bf16, tag="v1")
            nc.gpsimd.indirect_dma_start(
                out=v1[:], out_offset=None, in_=fbt2[:],
                in_offset=bass.IndirectOffsetOnAxis(ap=oi64[:, t:t + 1], axis=0),
            )
            # per-partition bilinear weights ride ScalarE's activation scale;
            # during the pipeline ramp (tiles 0-1) DVE is idle and ScalarE
            # gates the first apps, so run them as DVE tensor_scalar instead.
            sc = gpool.tile([128, 4, C], bf16, tag="sc")
            srcs = (v0[:, 0:C], v0[:, C:2 * C], v1[:, 0:C], v1[:, C:2 * C])
            for q in range(4):
                wcol = wq20[:, ST * q + t:ST * q + t + 1]
                if t < 2:
                    nc.vector.tensor_scalar_mul(sc[:, q], srcs[q], wcol)
                else:
                    nc.scalar.activation(out=sc[:, q], in_=srcs[q],
                                         func=Act.Copy, scale=wcol)
            acc = gpool.tile([128, C], bf16, tag="acc")
            tmp = gpool.tile([128, C], bf16, tag="tmp")
            nc.vector.tensor_add(out=tmp[:], in0=sc[:, 0], in1=sc[:, 1])
            nc.vector.tensor_add(out=acc[:], in0=sc[:, 2], in1=sc[:, 3])
            nc.vector.tensor_add(out=acc[:], in0=acc[:], in1=tmp[:])

            anchors = (2 * t, 2 * t + 1) if t < 4 else (8,)
            for a in anchors:
                half = (a % 2) * 64
                g0 = a * 2
                bt2 = ppool.tile([128, 2, K], f32, tag="btps")
                for chh in range(2):
                    # transpose as a plain matmul against a bf16 identity:
                    # bf16 moving operand, f32 PSUM out for the f32 b-path.
                    nc.tensor.matmul(
                        out=bt2[:, chh],
                        lhsT=acc[half:half + 64, chh * 128:(chh + 1) * 128],
                        rhs=id16[half:half + 64, :],
                        start=True, stop=True,
                    )
                dn2a = small.tile([128, 2, 2, K], f32, tag="dn2a")
                dn2b = small.tile([128, 2, 2, K], f32, tag="dn2b")
                # stagger: the pending anchor's app2 halves slot between this
                # anchor's app1s so dependent stages stay separated
                app(a16h_all[:, 64 * g0:64 * g0 + 64],
                    at16_all[:, 64 * g0:64 * g0 + 64],
                    bt2[:, 0], dn2a[:, 0], halves=2 if first_app else 1)
                first_app = False
                if pend_g is not None:
                    app(out1h_sb[:, pend_g], out116_sb[:, pend_g],
                        rb_sb[:, CT_OFF + 64 * pend_g:CT_OFF + 64 * pend_g + 64],
                        dn2b[:, 0])
                app(a16h_all[:, 64 * g0 + 64:64 * g0 + 128],
                    at16_all[:, 64 * g0 + 64:64 * g0 + 128],
                    bt2[:, 1], dn2a[:, 1])
                if pend_g is not None:
                    app(out1h_sb[:, pend_g + 1], out116_sb[:, pend_g + 1],
                        rb_sb[:, CT_OFF + 64 * pend_g + 64:
                              CT_OFF + 64 * pend_g + 128],
                        dn2b[:, 1])
                    o2p = small.tile([128, 2, K], f32, tag="o2p")
                    combine2(dn2b,
                             rb_sb[:, CT_OFF + 64 * pend_g:
                                   CT_OFF + 64 * pend_g + 128]
                             .rearrange("p (g k) -> p g k", g=2),
                             o2p[:], None, None)
                    nc.sync.dma_start(out=out_v[:, pend_g:pend_g + 2],
                                      in_=o2p[:])
                combine2(dn2a, bt2[:], out1_sb[:, g0:g0 + 2],
                         out116_sb[:, g0:g0 + 2], out1h_sb[:, g0:g0 + 2])
                pend_g = g0

        dn2b = small.tile([128, 2, 2, K], f32, tag="dn2b", name="dn2b_last")
        app(out1h_sb[:, pend_g], out116_sb[:, pend_g],
            rb_sb[:, CT_OFF + 64 * pend_g:CT_OFF + 64 * pend_g + 64],
            dn2b[:, 0], halves=2)
        app(out1h_sb[:, pend_g + 1], out116_sb[:, pend_g + 1],
            rb_sb[:, CT_OFF + 64 * pend_g + 64:CT_OFF + 64 * pend_g + 128],
            dn2b[:, 1], halves=2)
        o2p = small.tile([128, 2, K], f32, tag="o2p", name="o2p_last")
        combine2(dn2b,
                 rb_sb[:, CT_OFF + 64 * pend_g:CT_OFF + 64 * pend_g + 128]
                 .rearrange("p (g k) -> p g k", g=2),
                 o2p[:], None, None)
        nc.sync.dma_start(out=out_v[:, pend_g:pend_g + 2], in_=o2p[:])

        for p in (cpsum, ppool, small, ecpool, relpool, gpool, singles):
            p.release()

    if not nc.is_finalized():
        nc.finalize()
    return nc


def _host_prep(inputs):
    """Per-core input maps from the full inputs (pure layout transforms)."""
    import ml_dtypes

    ra = np.asarray(inputs["rois_feature_a"], dtype=np.float32).reshape(A, N, K, C)
    rc = np.asarray(inputs["rois_feature_c"], dtype=np.float32).reshape(A, N, K, C)
    fbf = np.asarray(inputs["feature_b"], dtype=np.float32)
    wr = np.asarray(inputs["W_reg"], dtype=np.float32)
    br = np.asarray(inputs["b_reg"], dtype=np.float32)

    # conv weights: [A, C, dy, dx] -> [c_lo, (c_hi dy dx), a] flat [128, 1152]
    w = wr.transpose(1, 2, 3, 0).reshape(2, 128, BS, BS, A)
    w = w.transpose(1, 0, 2, 3, 4).reshape(128, 128 * A)

    r = (0.5 * (BS - 1) + BS * np.arange(F)).astype(np.float32)
    xc_g = np.broadcast_to(r[None, :], (F, F))
    yc_g = np.ascontiguousarray(xc_g.T)
    pad = ST * 128 - NS
    xc_s = np.concatenate([np.broadcast_to(xc_g.reshape(1, K), (A, K)).reshape(NS),
                           np.full(pad, 31.5, np.float32)]).astype(np.float32)
    yc_s = np.concatenate([np.broadcast_to(yc_g.reshape(1, K), (A, K)).reshape(NS),
                           np.full(pad, 31.5, np.float32)]).astype(np.float32)

    def to_pt(v):  # [640] -> [128, 5]
        return np.ascontiguousarray(v.reshape(ST, 128).T)

    in_maps = []
    for n in range(N):
        fbw16 = np.zeros((128, NFB16E), ml_dtypes.bfloat16)
        fbw16[:, W_OFF:W_OFF + 1152] = w.astype(ml_dtypes.bfloat16)
        fbw16[0, B_OFF:B_OFF + A] = br.astype(ml_dtypes.bfloat16)
        fbw16[0, ONE_OFF:ONE_OFF + K] = 1.0
        fb_conv = fbf[n].reshape(C, F, BS, F, BS).transpose(0, 2, 4, 1, 3)
        fbw16[:, FB_OFF:] = (fb_conv.reshape(2, 128, 8192 // 2)
                             .transpose(1, 0, 2).reshape(128, 8192)
                             .astype(ml_dtypes.bfloat16))
        fbw_h = np.frombuffer(np.ascontiguousarray(fbw16).tobytes(),
                              dtype=np.float32).reshape(128, NFBW)

        a_t = ra[:, n].transpose(0, 2, 1).reshape(GT, 128, K)   # [(a c) k]
        c_t = rc[:, n].transpose(0, 2, 1).reshape(GT, 128, K)
        at_rows = np.ascontiguousarray(a_t.transpose(1, 0, 2).reshape(128, 1152))
        ct_rows = np.ascontiguousarray(c_t.transpose(1, 0, 2).reshape(128, 1152))
        a16_pack = np.frombuffer(at_rows.astype(ml_dtypes.bfloat16).tobytes(),
                                 dtype=np.float32).reshape(128, 576)
        a16h_pack = np.frombuffer(at_rows.astype(np.float16).tobytes(),
                                  dtype=np.float32).reshape(128, 576)

        rb_h = np.zeros((128, NRB), np.float32)
        rb_h[:, CT_OFF:CT_OFF + 1152] = ct_rows
        rb_h[:, A16_OFF:A16_OFF + 576] = a16_pack
        rb_h[:, A16H_OFF:A16H_OFF + 576] = a16h_pack
        rb_h[:, XC_OFF:XC_OFF + ST] = to_pt(xc_s)
        rb_h[:, YC_OFF:YC_OFF + ST] = to_pt(yc_s)
        rb_h[:, ID_OFF:ID_OFF + 128] = np.eye(128, dtype=np.float32)
        # anchor-pair selectors: regs10[:, t] dups in cols t and t+5
        for tt in range(2 * ST):
            ae = 2 * (tt % ST)
            rb_h[ae, SE_OFF + tt] = 1.0
            if ae + 1 < A:
                rb_h[ae + 1, SO_OFF + tt] = 1.0
        # bf16 64-block identity for the acc transpose matmuls
        id16 = np.zeros((128, 64), ml_dtypes.bfloat16)
        id16[np.arange(128), np.arange(128) % 64] = 1.0
        rb_h[:, ID16_OFF:ID16_OFF + 32] = np.frombuffer(
            np.ascontiguousarray(id16).tobytes(),
            dtype=np.float32).reshape(128, 32)

        # overlapping row-pair feature table: row r = channels of pixels
        # (r, r+1), so one gather fetches both x-neighbors of a sample.
        fb_flat = np.ascontiguousarray(fbf[n].reshape(C, H * W).T)
        fbt2_n = np.ascontiguousarray(
            np.concatenate([fb_flat[:-1], fb_flat[1:]], axis=1)
        ).astype(ml_dtypes.bfloat16)
        in_maps.append({"fbw": fbw_h, "rb": rb_h, "fbt2": fbt2_n})
    return in_maps


def _assemble(results):
    """Per-core 'out' [G, K] -> full [M, C, 1, 1]."""
    outs = []
    for n in range(N):
        o = np.asarray(results[n]["out"], dtype=np.float32).reshape(A, C, K)
        outs.append(o.transpose(0, 2, 1))            # [A, K, C]
    stk = np.stack(outs, axis=1)                      # [A, N, K, C]
    return np.ascontiguousarray(stk.reshape(M, C, 1, 1))


def kernel(**inputs):
    from concourse.bass_utils import run_bass_kernel_spmd

    if "nc" not in _CACHE:
        _CACHE["nc"] = _build_nc()
    nc = _CACHE["nc"]
    in_maps = _host_prep(inputs)
    res = run_bass_kernel_spmd(nc, in_maps, core_ids=list(range(N)))
    return _assemble(res.results)

